# revision 1
# baseline (speedup 1.0000x reference)
"""Trainium2 Bass kernel for nn_ClsHeader (octree pooling classifier head).

Data-parallel over the batch dimension: each of the 8 NeuronCores processes
one sample (its full octree subtree), weights are replicated, outputs are
gathered host-side.  No collectives needed.

Per-core pipeline:
  - data0 [32768,128] (depth 5): four 4MB DMA loads (alternating between the
    SP and ACT HWDGE rings) with partition = depth-3 subtree (64 contiguous
    rows = 32KB per partition); per-partition max-pool as an IN-PLACE
    contiguous tensor-tensor max tree (4096->...->128, 6 DVE ops; ~0.9
    ns/elem vs ~1.5 ns/elem for the strided reduce) -> [128 d3-nodes,
    128 ch]; PE transposes collect all four tiles into one PSUM bank; one
    final contiguous 8->1 reduce -> x0 [128 ch, 64 d2-nodes].
  - data1 [4096,128] (depth 4): partition = d3 node (8 rows); 3-round
    in-place TT max tree over the sibling axis, 4 PE transposes, 8->1 reduce.
  - data2 [512,128] (depth 3): partition = d2 node (8 rows), 3-round tree.
  - Conv1x1+BN folded host-side into W' = conv_w*inv, b' = beta-mean*inv.
    y^T = W'^T @ x^T as 24 accumulating matmuls (3 K-chunks x 8 M-chunks)
    into two PSUM banks (groups sequential per bank; ScalarE drains one bank
    while PE fills the other).
  - ScalarE activation(Relu, bias=b', accum_out) fuses bias+relu+node-sum.
  - Head matmul: logit[1,40] = sum_m s_m^T @ (head_w/64)_m + head_b; the
    output DMA goes via GPSIMD so the SP ring never blocks on compute.

The walrus build here accepts only one sync-wait per instruction, so
_split_multiwaits() rewrites the scheduled program, moving extra waits onto
single-wait NOPs.  kernel() runs through a cached jitted shard_map executor
(the same custom-call path run_bass_kernel_spmd uses under axon) so repeated
calls do not re-trace or re-compile.

Measured (repeat-slope, drift-cancelled): loads-only ~49.5us/rep (18.25 MiB
at the ~380 GB/s per-core DMA aggregate cap -- 1/2/3 rings and SWDGE all
measure the same), compute-only ~35us/rep, full ~50.5us/rep: the kernel is
DMA-bound at the memory roofline.  The previous strided-reduce pooling ran
~59.6us of DVE work (strided reduce_max streams ~1.5 cycle/elem vs ~0.9
for contiguous tensor-tensor); the in-place max tree moved the bottleneck
to the DMA floor.  gpsimd cannot help pool (walrus rejects TensorTensor on
Pool; it is a software DSP) and ACT has no max op, so all pooling is DVE.
"""

import os
import sys

for _p in ("/opt/trn_rl_repo", "/root/.axon_site/_ro/trn_rl_repo"):
    if os.path.isdir(_p) and _p not in sys.path:
        sys.path.append(_p)

import numpy as np

import concourse.bass as bass
import concourse.mybir as mybir
import concourse.tile as tile
from concourse.bass_utils import run_bass_kernel_spmd
from concourse.masks import make_identity

F32 = mybir.dt.float32
N_CORES = 8
D0, D1, D2 = 32768, 4096, 512  # per-core (per-sample) rows at depths 5/4/3
C = 128  # channels per input level
OUTC = 1024  # conv output channels
NCLS = 40
N2 = 64  # depth-2 nodes per sample
AX = mybir.AxisListType.X


def _split_multiwaits(nc):
    """The walrus build in this container accepts only ONE sync-wait per
    instruction; move extra waits onto dedicated NOPs inserted just before
    the owning instruction (same engine, so sequencer order is preserved)."""
    n_split = 0
    for f in nc.m.functions:
        for bb in f.blocks:
            out = []
            changed = False
            for inst in bb.instructions:
                si = inst.sync_info
                waits = list(si.on_wait) if si is not None else []
                if len(waits) > 1:
                    for j, w in enumerate(waits[:-1]):
                        nop = mybir.InstNoOp(
                            name=f"{inst.name}-wsplit{j}", ins=[], outs=[]
                        )
                        nop.engine = inst.engine
                        nop.sync_info = mybir.SyncInfo(on_wait=[w], on_update=[])
                        out.append(nop)
                    si.on_wait = [waits[-1]]
                    changed = True
                    n_split += 1
                out.append(inst)
            if changed:
                bb.instructions = out
    return n_split


def _build_nc(split=True, repeat=1, mode="full", d0_split="all", ld_bufs=5):
    # mode: "full" | "dma" (loads only, no compute) | "compute" (no big loads)
    # d0_split: "all" = every d0 tile halved across both rings;
    #           "first" = only tile 0 halved, others alternate rings.
    nc = bass.Bass("TRN2", num_devices=N_CORES)
    d0 = nc.dram_tensor("d0", [D0, C], F32, kind="ExternalInput")
    d1 = nc.dram_tensor("d1", [D1, C], F32, kind="ExternalInput")
    d2 = nc.dram_tensor("d2", [D2, C], F32, kind="ExternalInput")
    wT = nc.dram_tensor("wT", [128, 3 * OUTC], F32, kind="ExternalInput")
    bias8 = nc.dram_tensor("bias8", [128, 8], F32, kind="ExternalInput")
    hw8 = nc.dram_tensor("hw8", [128, 8 * NCLS], F32, kind="ExternalInput")
    hb = nc.dram_tensor("hb", [1, NCLS], F32, kind="ExternalInput")
    out = nc.dram_tensor("out", [1, NCLS], F32, kind="ExternalOutput")

    with tile.TileContext(nc) as tc:
        with (
            tc.tile_pool(name="consts", bufs=1) as consts,
            tc.tile_pool(name="inp", bufs=4) as inp,
            tc.tile_pool(name="work", bufs=1) as work,
            tc.tile_pool(name="pt", bufs=4, space="PSUM") as pt,
            tc.tile_pool(name="py", bufs=1, space="PSUM") as py,
        ):
            wT_s = consts.tile([128, 3 * OUTC], F32)
            nc.scalar.dma_start(out=wT_s[:], in_=wT[:])
            bias8_s = consts.tile([128, 8], F32)
            nc.scalar.dma_start(out=bias8_s[:], in_=bias8[:])
            hw8_s = consts.tile([128, 8 * NCLS], F32)
            nc.scalar.dma_start(out=hw8_s[:], in_=hw8[:])
            hb_s = consts.tile([1, NCLS], F32)
            nc.scalar.dma_start(out=hb_s[:], in_=hb[:])
            ident = consts.tile([128, 128], F32)
            make_identity(nc, ident[:])
            ones1 = consts.tile([1, 1], F32)
            nc.vector.memset(ones1[:], 1.0)
            # warm-up Relu so the ACT table DMA (~2.7us) overlaps the ramp
            # instead of sitting in front of the first real activation
            actwarm = consts.tile([1, 1], F32)
            nc.scalar.activation(
                actwarm[:], ones1[:], mybir.ActivationFunctionType.Relu
            )

            if mode == "compute":
                ld1c = consts.tile([128, 4, 8, C], F32)
                nc.vector.memset(ld1c[:], 0.25)
                ld2c = consts.tile([64, 8, C], F32)
                nc.vector.memset(ld2c[:], 0.25)
                ldcs = []
                for t in range(4):
                    ldc = consts.tile([128, 64, C], F32, name=f"ldc{t}")
                    nc.vector.memset(ldc[:], 0.25)
                    ldcs.append(ldc)

            for _rep in range(repeat):
              # bufs=2 so rep N+1's producers never stall behind rep N's
              # last consumer (a stalled ACT instruction would also block
              # the ACT-ring DMA configs queued behind it).
              x0 = work.tile([128, N2], F32, tag="x0", bufs=2)
              x1 = work.tile([128, N2], F32, tag="x1", bufs=2)
              x2 = work.tile([128, N2], F32, tag="x2", bufs=2)
              stile = work.tile([128, 8], F32, tag="stile", bufs=2)
              yscr = work.tile([128, 64], F32, tag="yscr", bufs=2)
              outs = work.tile([1, NCLS], F32, tag="outs", bufs=2)

              # PSUM: one bank collects all four data0 transposes, one bank
              # the four data1 transposes, one the data2 transpose; plus two
              # banks for the conv output (so ScalarE drains one while PE
              # fills the other) and one for the head logits = 6 of 8 banks.
              tpb0 = pt.tile([128, 512], F32, tag="tpb0", bufs=1)
              tpb1 = pt.tile([128, 512], F32, tag="tpb1", bufs=1)
              psum_ya = py.tile([128, 256], F32, tag="pya")
              psum_yb = py.tile([128, 256], F32, tag="pyb")
              psum_l = py.tile([1, NCLS], F32, tag="pl")

              # ---- data0: partition = d3 subtree (64 contiguous rows) ----
              # Every tile is split across the SP and ACT HWDGE rings so
              # tiles complete in order at an even ~11us cadence (keeps the
              # DVE tree pipeline fed and both rings balanced end-to-end).
              d0v = d0[:].rearrange("(t p k) c -> t p k c", t=4, p=128, k=64)
              ld0s = []
              for t in range(4):
                  ld = ldcs[t] if mode == "compute" else inp.tile(
                      [128, 64, C], F32, bufs=ld_bufs, tag="ld"
                  )
                  if mode != "compute":
                      if d0_split == "all" or t == 0:
                          nc.sync.dma_start(
                              out=ld[:, 0:32, :], in_=d0v[t][:, 0:32, :]
                          )
                          nc.scalar.dma_start(
                              out=ld[:, 32:64, :], in_=d0v[t][:, 32:64, :]
                          )
                      else:
                          eng = nc.sync if t % 2 == 0 else nc.scalar
                          eng.dma_start(out=ld[:], in_=d0v[t])
                  ld0s.append(ld)

              # ---- data1 / data2 loads ----
              ld1 = ld1c if mode == "compute" else inp.tile(
                  [128, 4, 8, C], F32, bufs=1, tag="ld1"
              )
              ld2 = ld2c if mode == "compute" else inp.tile(
                  [64, 8, C], F32, bufs=1, tag="ld2"
              )
              if mode != "compute":
                  d1v = d1[:].rearrange("(j p k) c -> p j k c", j=4, p=128, k=8)
                  nc.sync.dma_start(out=ld1[:, 0:2], in_=d1v[:, 0:2])
                  nc.scalar.dma_start(out=ld1[:, 2:4], in_=d1v[:, 2:4])
                  nc.scalar.dma_start(
                      out=ld2[:], in_=d2[:].rearrange("(p k) c -> p k c", p=64, k=8)
                  )

              if mode == "dma":
                  dummy = work.tile([1, 8], F32, tag="dummy")
                  for t in range(4):
                      nc.vector.tensor_copy(
                          dummy[:, t : t + 1], ld0s[t][0:1, 0, 0:1]
                      )
                  nc.vector.tensor_copy(dummy[:, 4:5], ld1[0:1, 0, 0, 0:1])
                  nc.vector.tensor_copy(dummy[:, 5:6], ld2[0:1, 0, 0:1])
                  nc.vector.tensor_copy(outs[:, 0:8], dummy[:, 0:8])
                  nc.gpsimd.dma_start(out=out[:], in_=outs[:])
                  continue

              # stage-1 pool: in-place contiguous tensor-tensor max tree per
              # tile (64 sibling rows -> 1 per partition), then PE transpose
              # and a single contiguous 8:1 reduce over d3 siblings.
              for t in range(4):
                  ldf = ld0s[t][:].rearrange("p k c -> p (k c)")
                  n = 4096
                  while n >= C:
                      nc.vector.tensor_tensor(
                          out=ldf[:, 0:n],
                          in0=ldf[:, 0:n],
                          in1=ldf[:, n : 2 * n],
                          op=mybir.AluOpType.max,
                      )
                      n //= 2
                  nc.tensor.transpose(
                      tpb0[:, t * 128 : (t + 1) * 128], ld0s[t][:, 0, :], ident[:]
                  )
              nc.vector.reduce_max(
                  x0[:], tpb0[:].rearrange("p (g k) -> p g k", k=8), axis=AX
              )

              # ---- data1: partition = d3 node (8 contiguous rows) ----
              for sh in (4, 2, 1):
                  nc.vector.tensor_tensor(
                      out=ld1[:, :, 0:sh, :],
                      in0=ld1[:, :, 0:sh, :],
                      in1=ld1[:, :, sh : 2 * sh, :],
                      op=mybir.AluOpType.max,
                  )
              for j in range(4):
                  nc.tensor.transpose(
                      tpb1[:, j * 128 : (j + 1) * 128], ld1[:, j, 0, :], ident[:]
                  )
              nc.vector.reduce_max(
                  x1[:], tpb1[:].rearrange("p (g k) -> p g k", k=8), axis=AX
              )

              # ---- data2: partition = d2 node (8 contiguous rows) ----
              for sh in (4, 2, 1):
                  nc.vector.tensor_tensor(
                      out=ld2[:, 0:sh, :],
                      in0=ld2[:, 0:sh, :],
                      in1=ld2[:, sh : 2 * sh, :],
                      op=mybir.AluOpType.max,
                  )
              tp2 = pt.tile([128, 64], F32, tag="tp", bufs=1)
              nc.tensor.transpose(tp2[:], ld2[:, 0, :], ident[0:64, 0:64])
              nc.scalar.copy(x2[:], tp2[:])

              if mode == "pool":
                  nc.vector.tensor_copy(outs[:, 0:1], x0[0:1, 0:1])
                  nc.vector.tensor_copy(outs[:, 1:2], x1[0:1, 0:1])
                  nc.vector.tensor_copy(outs[:, 2:3], x2[0:1, 0:1])
                  nc.gpsimd.dma_start(out=out[:], in_=outs[:])
                  continue

              # Conv matmuls: 8 m-chunks x 3 K-chunks; accumulation groups
              # run back-to-back within each bank.  ScalarE activation
              # (bias + relu + node-sum via accum_out) drains each slice.
              for m in range(8):
                  # parity-interleaved banks: ScalarE drains slice m in one
                  # bank while PE runs group m+1's matmuls in the other
                  pybank = psum_ya if m % 2 == 0 else psum_yb
                  sl = pybank[:, (m // 2) * 64 : (m // 2 + 1) * 64]
                  for ki, xk in ((0, x0), (1, x1), (2, x2)):
                      nc.tensor.matmul(
                          sl,
                          wT_s[:, ki * OUTC + m * 128 : ki * OUTC + (m + 1) * 128],
                          xk[:],
                          start=(ki == 0),
                          stop=(ki == 2),
                      )
                  nc.scalar.activation(
                      yscr[:],
                      sl,
                      mybir.ActivationFunctionType.Relu,
                      bias=bias8_s[:, m : m + 1],
                      scale=1.0,
                      accum_out=stile[:, m : m + 1],
                  )

              for m in range(8):
                  nc.tensor.matmul(
                      psum_l[:],
                      stile[:, m : m + 1],
                      hw8_s[:, m * NCLS : (m + 1) * NCLS],
                      start=(m == 0),
                      stop=False,
                  )
              # head_b folded in as a rank-1 (K=1) accumulation: ones.T @ hb
              nc.tensor.matmul(
                  psum_l[:], ones1[:], hb_s[:], start=False, stop=True
              )
              nc.scalar.copy(outs[:], psum_l[:])
              nc.gpsimd.dma_start(out=out[:], in_=outs[:])

    if split:
        _split_multiwaits(nc)
    return nc


_NC = None


def _get_nc():
    global _NC
    if _NC is None:
        _NC = _build_nc()
    return _NC


def make_in_maps(
    data0, data1, data2, conv_w, bn_gamma, bn_beta, bn_mean, bn_var, head_w, head_b
):
    f = np.float32
    data0 = np.ascontiguousarray(data0, dtype=f)
    data1 = np.ascontiguousarray(data1, dtype=f)
    data2 = np.ascontiguousarray(data2, dtype=f)
    conv_w = np.asarray(conv_w, dtype=f)
    bn_gamma = np.asarray(bn_gamma, dtype=f)
    bn_beta = np.asarray(bn_beta, dtype=f)
    bn_mean = np.asarray(bn_mean, dtype=f)
    bn_var = np.asarray(bn_var, dtype=f)
    head_w = np.asarray(head_w, dtype=f)
    head_b = np.asarray(head_b, dtype=f)

    inv = (bn_gamma / np.sqrt(bn_var + np.float32(1e-5))).astype(f)
    w_folded = (conv_w * inv[None, :]).astype(f)  # [384, 1024]
    b_folded = (bn_beta - bn_mean * inv).astype(f)  # [1024]

    # wT[p, k*1024+j] = W'[k*128+p, j]  (K-chunk-major along free dim)
    wT = np.ascontiguousarray(
        w_folded.reshape(3, 128, OUTC).transpose(1, 0, 2).reshape(128, 3 * OUTC)
    )
    # bias8[p, m] = b'[m*128+p]
    bias8 = np.ascontiguousarray(b_folded.reshape(8, 128).T)
    # hw8[p, m*40+q] = head_w[m*128+p, q] / 64   (1/64 folds the mean-pool)
    hw8 = np.ascontiguousarray(
        (head_w / np.float32(N2)).reshape(8, 128, NCLS).transpose(1, 0, 2).reshape(128, 8 * NCLS)
    )
    hb = np.ascontiguousarray(head_b.reshape(1, NCLS))

    in_maps = []
    for c in range(N_CORES):
        in_maps.append(
            {
                "d0": data0[c * D0 : (c + 1) * D0],
                "d1": data1[c * D1 : (c + 1) * D1],
                "d2": data2[c * D2 : (c + 1) * D2],
                "wT": wT,
                "bias8": bias8,
                "hw8": hw8,
                "hb": hb,
            }
        )
    return in_maps


_RUNNER = None


def _make_runner(nc):
    """Jitted SPMD executor (mirrors bass2jax.run_bass_via_pjrt but reuses
    one jit so repeated calls don't re-trace/re-compile)."""
    if True:
        import jax
        from jax.experimental.shard_map import shard_map
        from jax.sharding import Mesh, PartitionSpec

        from concourse import bass2jax, mybir as mb

        bass2jax.install_neuronx_cc_hook()
        partition_name = (
            nc.partition_id_tensor.name if nc.partition_id_tensor else None
        )
        in_names, out_names, out_avals, zero_outs = [], [], [], []
        for alloc in nc.m.functions[0].allocations:
            if not isinstance(alloc, mb.MemoryLocationSet):
                continue
            name = alloc.memorylocations[0].name
            if alloc.kind == "ExternalInput":
                if name != partition_name:
                    in_names.append(name)
            elif alloc.kind == "ExternalOutput":
                out_names.append(name)
                shape = tuple(alloc.tensor_shape)
                dtype = mb.dt.np(alloc.dtype)
                out_avals.append(jax.core.ShapedArray(shape, dtype))
                zero_outs.append(np.zeros(shape, dtype))
        n_params = len(in_names)
        all_in_names = in_names + out_names
        if partition_name is not None:
            all_in_names = all_in_names + [partition_name]

        def _body(*args):
            operands = list(args)
            if partition_name is not None:
                operands.append(bass2jax.partition_id_tensor())
            outs = bass2jax._bass_exec_p.bind(
                *operands,
                out_avals=tuple(out_avals),
                in_names=tuple(all_in_names),
                out_names=tuple(out_names),
                lowering_input_output_aliases=(),
                sim_require_finite=True,
                sim_require_nnan=True,
                nc=nc,
            )
            return tuple(outs)

        devices = jax.devices()[:N_CORES]
        mesh = Mesh(np.asarray(devices), ("core",))
        n_outs = len(out_avals)
        in_specs = (PartitionSpec("core"),) * (n_params + n_outs)
        out_specs = (PartitionSpec("core"),) * n_outs
        # No donation: the kernel writes every element of "out", so the
        # zero placeholder inputs can live on device and be reused.
        sharded = jax.jit(
            shard_map(
                _body,
                mesh=mesh,
                in_specs=in_specs,
                out_specs=out_specs,
                check_rep=False,
            ),
            keep_unused=True,
        )
        return dict(
            nc=nc,
            sharded=sharded,
            in_names=in_names,
            out_names=out_names,
            out_avals=out_avals,
            zero_outs=zero_outs,
            mesh=mesh,
        )


def _get_runner():
    global _RUNNER
    if _RUNNER is None:
        _RUNNER = _make_runner(_get_nc())
    return _RUNNER


def _concat_inputs(r, in_maps):
    return [
        np.concatenate([np.asarray(m[name]) for m in in_maps], axis=0)
        for name in r["in_names"]
    ]


def _concat_zeros(r):
    return [
        np.zeros((N_CORES * z.shape[0], *z.shape[1:]), z.dtype)
        for z in r["zero_outs"]
    ]


def _run(r, concat_in, concat_zeros=None):
    if concat_zeros is None:
        concat_zeros = _concat_zeros(r)
    out_arrs = r["sharded"](*concat_in, *concat_zeros)
    return out_arrs


def kernel(**inputs) -> np.ndarray:
    r = _get_runner()
    in_maps = make_in_maps(**inputs)
    out_arrs = _run(r, _concat_inputs(r, in_maps))
    return np.asarray(out_arrs[r["out_names"].index("out")])


def device_place_and_time(r, inputs, iters=20, batches=4):
    """Pre-place inputs on device, then time batches of back-to-back
    dispatches.  Returns (per-call seconds list, out array)."""
    import time

    import jax
    from jax.sharding import NamedSharding, PartitionSpec

    sharding = NamedSharding(r["mesh"], PartitionSpec("core"))
    concat_in = _concat_inputs(r, make_in_maps(**inputs))
    dev_in = [jax.device_put(a, sharding) for a in concat_in]
    dev_zeros = [jax.device_put(z, sharding) for z in _concat_zeros(r)]
    out_arrs = _run(r, dev_in, dev_zeros)  # warm
    out = np.asarray(out_arrs[r["out_names"].index("out")])
    times = []
    for _ in range(batches):
        t0 = time.perf_counter()
        last = None
        for _ in range(iters):
            last = _run(r, dev_in, dev_zeros)
        jax.block_until_ready(last)
        t1 = time.perf_counter()
        times.append((t1 - t0) / iters)
    return times, out



# revision 13
# speedup vs baseline: 1.9655x; 1.9655x over previous
"""Trainium2 Bass kernel for nn_ClsHeader (octree pooling classifier head).

Data-parallel over the batch dimension: each of the 8 NeuronCores processes
one sample (its full octree subtree), weights are replicated, outputs are
gathered host-side.  No collectives needed.

Per-core pipeline:
  - data0 [32768,128] (depth 5): four 4MB DMA loads (alternating between the
    SP and ACT HWDGE rings) with partition = depth-3 subtree (64 contiguous
    rows = 32KB per partition); per-partition max-pool as an IN-PLACE
    contiguous tensor-tensor max tree (4096->...->128, 6 DVE ops; ~0.9
    ns/elem vs ~1.5 ns/elem for the strided reduce) -> [128 d3-nodes,
    128 ch]; PE transposes collect all four tiles into one PSUM bank; one
    final contiguous 8->1 reduce -> x0 [128 ch, 64 d2-nodes].
  - data1 [4096,128] (depth 4): partition = d3 node (8 rows); 3-round
    in-place TT max tree over the sibling axis, 4 PE transposes, 8->1 reduce.
  - data2 [512,128] (depth 3): partition = d2 node (8 rows), 3-round tree.
  - Conv1x1+BN folded host-side into W' = conv_w*inv, b' = beta-mean*inv.
    y^T = W'^T @ x^T as 24 accumulating matmuls (3 K-chunks x 8 M-chunks)
    into two PSUM banks (groups sequential per bank; ScalarE drains one bank
    while PE fills the other).
  - ScalarE activation(Relu, bias=b', accum_out) fuses bias+relu+node-sum.
  - Head matmul: logit[1,40] = sum_m s_m^T @ (head_w/64)_m + head_b; the
    output DMA goes via GPSIMD so the SP ring never blocks on compute.

The walrus build here accepts only one sync-wait per instruction, so
_split_multiwaits() rewrites the scheduled program, moving extra waits onto
single-wait NOPs.  kernel() runs through a cached jitted shard_map executor
(the same custom-call path run_bass_kernel_spmd uses under axon) so repeated
calls do not re-trace or re-compile.

Precision: the three data tensors, the pooling tree, the transposes and the
conv weights run in bf16 (host converts; rel-err vs the fp32 reference is
~1.4e-3, well inside the 2e-2 gate).  This halves the dominant HBM traffic
(18.25 MiB -> 9.13 MiB per core) AND doubles DVE pooling throughput: DVE
tensor_tensor hits the 2x_1P perf mode only for 2-byte dtypes with step-1
access (cost ~ 58 + FD/2 cycles @ 0.96 GHz), so the bf16 max tree runs
~0.52 ns/elem vs ~1.04 fp32.  Per-rep engine budgets: DMA ~25us (9.57 MB
at the ~380 GB/s per-core aggregate cap), DVE ~23us, PE/ACT tiny -- the
kernel stays DMA-bound at the (halved) memory roofline.  uint8/fp8 would
halve DMA again but DVE has no 8-bit packing (1x mode, ~38us of pooling),
so 16-bit is the sweet spot.  gpsimd cannot help pool (walrus rejects
TensorTensor on Pool; it is a software DSP) and ACT has no max op, so all
pooling is DVE.  BN is folded into the conv weights host-side; the conv
accumulates in fp32 PSUM and bias/relu/node-sum/head stay fp32.

Measured fp32 baseline (repeat-slope, drift-cancelled): loads-only
~49.5us/rep, compute-only ~35us/rep, full ~50.5us/rep (DMA-bound).
"""

import os
import sys

for _p in ("/opt/trn_rl_repo", "/root/.axon_site/_ro/trn_rl_repo"):
    if os.path.isdir(_p) and _p not in sys.path:
        sys.path.append(_p)

import numpy as np

import concourse.bass as bass
import concourse.mybir as mybir
import concourse.tile as tile
from concourse.bass_utils import run_bass_kernel_spmd
from concourse.masks import make_identity

F32 = mybir.dt.float32
BF16 = mybir.dt.bfloat16
N_CORES = 8
D0, D1, D2 = 32768, 4096, 512  # per-core (per-sample) rows at depths 5/4/3
C = 128  # channels per input level
OUTC = 1024  # conv output channels
NCLS = 40
N2 = 64  # depth-2 nodes per sample
AX = mybir.AxisListType.X


def _split_multiwaits(nc):
    """The walrus build in this container accepts only ONE sync-wait per
    instruction; move extra waits onto dedicated NOPs inserted just before
    the owning instruction (same engine, so sequencer order is preserved)."""
    n_split = 0
    for f in nc.m.functions:
        for bb in f.blocks:
            out = []
            changed = False
            for inst in bb.instructions:
                si = inst.sync_info
                waits = list(si.on_wait) if si is not None else []
                if len(waits) > 1:
                    for j, w in enumerate(waits[:-1]):
                        nop = mybir.InstNoOp(
                            name=f"{inst.name}-wsplit{j}", ins=[], outs=[]
                        )
                        nop.engine = inst.engine
                        nop.sync_info = mybir.SyncInfo(on_wait=[w], on_update=[])
                        out.append(nop)
                    si.on_wait = [waits[-1]]
                    changed = True
                    n_split += 1
                out.append(inst)
            if changed:
                bb.instructions = out
    return n_split


def _build_nc(split=True, repeat=1, mode="full", d0_split="all", ld_bufs=5):
    # mode: "full" | "dma" (loads only, no compute) | "compute" (no big loads)
    # d0_split: "all" = every d0 tile halved across both rings;
    #           "first" = only tile 0 halved, others alternate rings.
    nc = bass.Bass("TRN2", num_devices=N_CORES)
    d0 = nc.dram_tensor("d0", [D0, C], BF16, kind="ExternalInput")
    d1 = nc.dram_tensor("d1", [D1, C], BF16, kind="ExternalInput")
    d2 = nc.dram_tensor("d2", [D2, C], BF16, kind="ExternalInput")
    wT = nc.dram_tensor("wT", [128, 3 * OUTC], BF16, kind="ExternalInput")
    bias8 = nc.dram_tensor("bias8", [128, 8], F32, kind="ExternalInput")
    hw8 = nc.dram_tensor("hw8", [128, 8 * NCLS], F32, kind="ExternalInput")
    hb = nc.dram_tensor("hb", [1, NCLS], F32, kind="ExternalInput")
    out = nc.dram_tensor("out", [1, NCLS], F32, kind="ExternalOutput")

    with tile.TileContext(nc) as tc:
        with (
            tc.tile_pool(name="consts", bufs=1) as consts,
            tc.tile_pool(name="inp", bufs=4) as inp,
            tc.tile_pool(name="work", bufs=1) as work,
            tc.tile_pool(name="pt", bufs=4, space="PSUM") as pt,
            tc.tile_pool(name="py", bufs=1, space="PSUM") as py,
        ):
            wT_s = consts.tile([128, 3 * OUTC], BF16)
            nc.scalar.dma_start(out=wT_s[:], in_=wT[:])
            bias8_s = consts.tile([128, 8], F32)
            nc.scalar.dma_start(out=bias8_s[:], in_=bias8[:])
            hw8_s = consts.tile([128, 8 * NCLS], F32)
            nc.scalar.dma_start(out=hw8_s[:], in_=hw8[:])
            hb_s = consts.tile([1, NCLS], F32)
            nc.scalar.dma_start(out=hb_s[:], in_=hb[:])
            ident = consts.tile([128, 128], BF16)
            make_identity(nc, ident[:])
            ones1 = consts.tile([1, 1], F32)
            nc.vector.memset(ones1[:], 1.0)
            # warm-up Relu so the ACT table DMA (~2.7us) overlaps the ramp
            # instead of sitting in front of the first real activation
            actwarm = consts.tile([1, 1], F32)
            nc.scalar.activation(
                actwarm[:], ones1[:], mybir.ActivationFunctionType.Relu
            )

            if mode == "compute":
                ld1c = consts.tile([128, 4, 8, C], BF16)
                nc.vector.memset(ld1c[:], 0.25)
                ld2c = consts.tile([64, 8, C], BF16)
                nc.vector.memset(ld2c[:], 0.25)
                ldcs = []
                for t in range(4):
                    ldc = consts.tile([128, 64, C], BF16, name=f"ldc{t}")
                    nc.vector.memset(ldc[:], 0.25)
                    ldcs.append(ldc)

            for _rep in range(repeat):
              # bufs=2 so rep N+1's producers never stall behind rep N's
              # last consumer (a stalled ACT instruction would also block
              # the ACT-ring DMA configs queued behind it).
              x0 = work.tile([128, N2], BF16, tag="x0", bufs=2)
              x1 = work.tile([128, N2], BF16, tag="x1", bufs=2)
              x2 = work.tile([128, N2], BF16, tag="x2", bufs=2)
              stile = work.tile([128, 8], F32, tag="stile", bufs=2)
              yscr = work.tile([128, 64], F32, tag="yscr", bufs=2)
              outs = work.tile([1, NCLS], F32, tag="outs", bufs=2)

              # PSUM: one bank collects all four data0 transposes, one bank
              # the four data1 transposes, one the data2 transpose; plus two
              # banks for the conv output (so ScalarE drains one while PE
              # fills the other) and one for the head logits = 6 of 8 banks.
              tpb0 = pt.tile([128, 512], BF16, tag="tpb0", bufs=1)
              tpb1 = pt.tile([128, 512], BF16, tag="tpb1", bufs=1)
              psum_ya = py.tile([128, 256], F32, tag="pya")
              psum_yb = py.tile([128, 256], F32, tag="pyb")
              psum_l = py.tile([1, NCLS], F32, tag="pl")

              # ---- data0: partition = d3 subtree (64 contiguous rows) ----
              # Every tile is split across the SP and ACT HWDGE rings so
              # tiles complete in order at an even ~11us cadence (keeps the
              # DVE tree pipeline fed and both rings balanced end-to-end).
              d0v = d0[:].rearrange("(t p k) c -> t p k c", t=4, p=128, k=64)
              ld0s = []
              for t in range(4):
                  ld = ldcs[t] if mode == "compute" else inp.tile(
                      [128, 64, C], BF16, bufs=ld_bufs, tag="ld"
                  )
                  if mode != "compute":
                      if d0_split == "all" or t == 0:
                          nc.sync.dma_start(
                              out=ld[:, 0:32, :], in_=d0v[t][:, 0:32, :]
                          )
                          nc.scalar.dma_start(
                              out=ld[:, 32:64, :], in_=d0v[t][:, 32:64, :]
                          )
                      else:
                          eng = nc.sync if t % 2 == 0 else nc.scalar
                          eng.dma_start(out=ld[:], in_=d0v[t])
                  ld0s.append(ld)

              # ---- data1 / data2 loads ----
              ld1 = ld1c if mode == "compute" else inp.tile(
                  [128, 4, 8, C], BF16, bufs=1, tag="ld1"
              )
              ld2 = ld2c if mode == "compute" else inp.tile(
                  [64, 8, C], BF16, bufs=1, tag="ld2"
              )
              if mode != "compute":
                  d1v = d1[:].rearrange("(j p k) c -> p j k c", j=4, p=128, k=8)
                  nc.sync.dma_start(out=ld1[:, 0:2], in_=d1v[:, 0:2])
                  nc.scalar.dma_start(out=ld1[:, 2:4], in_=d1v[:, 2:4])
                  nc.scalar.dma_start(
                      out=ld2[:], in_=d2[:].rearrange("(p k) c -> p k c", p=64, k=8)
                  )

              if mode == "dma":
                  dummy = work.tile([1, 8], F32, tag="dummy")
                  for t in range(4):
                      nc.vector.tensor_copy(
                          dummy[:, t : t + 1], ld0s[t][0:1, 0, 0:1]
                      )
                  nc.vector.tensor_copy(dummy[:, 4:5], ld1[0:1, 0, 0, 0:1])
                  nc.vector.tensor_copy(dummy[:, 5:6], ld2[0:1, 0, 0:1])
                  nc.vector.tensor_copy(outs[:, 0:8], dummy[:, 0:8])
                  nc.gpsimd.dma_start(out=out[:], in_=outs[:])
                  continue

              # stage-1 pool: in-place contiguous tensor-tensor max tree per
              # tile (64 sibling rows -> 1 per partition), then PE transpose
              # and a single contiguous 8:1 reduce over d3 siblings.
              for t in range(4):
                  ldf = ld0s[t][:].rearrange("p k c -> p (k c)")
                  n = 4096
                  while n >= C:
                      nc.vector.tensor_tensor(
                          out=ldf[:, 0:n],
                          in0=ldf[:, 0:n],
                          in1=ldf[:, n : 2 * n],
                          op=mybir.AluOpType.max,
                      )
                      n //= 2
                  nc.tensor.transpose(
                      tpb0[:, t * 128 : (t + 1) * 128], ld0s[t][:, 0, :], ident[:]
                  )
              nc.vector.reduce_max(
                  x0[:], tpb0[:].rearrange("p (g k) -> p g k", k=8), axis=AX
              )

              # ---- data1: partition = d3 node (8 contiguous rows) ----
              for sh in (4, 2, 1):
                  nc.vector.tensor_tensor(
                      out=ld1[:, :, 0:sh, :],
                      in0=ld1[:, :, 0:sh, :],
                      in1=ld1[:, :, sh : 2 * sh, :],
                      op=mybir.AluOpType.max,
                  )
              for j in range(4):
                  nc.tensor.transpose(
                      tpb1[:, j * 128 : (j + 1) * 128], ld1[:, j, 0, :], ident[:]
                  )
              nc.vector.reduce_max(
                  x1[:], tpb1[:].rearrange("p (g k) -> p g k", k=8), axis=AX
              )

              # ---- data2: partition = d2 node (8 contiguous rows) ----
              for sh in (4, 2, 1):
                  nc.vector.tensor_tensor(
                      out=ld2[:, 0:sh, :],
                      in0=ld2[:, 0:sh, :],
                      in1=ld2[:, sh : 2 * sh, :],
                      op=mybir.AluOpType.max,
                  )
              tp2 = pt.tile([128, 64], BF16, tag="tp", bufs=1)
              nc.tensor.transpose(tp2[:], ld2[:, 0, :], ident[0:64, 0:64])
              nc.scalar.copy(x2[:], tp2[:])

              if mode == "pool":
                  nc.vector.tensor_copy(outs[:, 0:1], x0[0:1, 0:1])
                  nc.vector.tensor_copy(outs[:, 1:2], x1[0:1, 0:1])
                  nc.vector.tensor_copy(outs[:, 2:3], x2[0:1, 0:1])
                  nc.gpsimd.dma_start(out=out[:], in_=outs[:])
                  continue

              # Conv matmuls: 8 m-chunks x 3 K-chunks; accumulation groups
              # run back-to-back within each bank.  ScalarE activation
              # (bias + relu + node-sum via accum_out) drains each slice.
              for m in range(8):
                  # parity-interleaved banks: ScalarE drains slice m in one
                  # bank while PE runs group m+1's matmuls in the other
                  pybank = psum_ya if m % 2 == 0 else psum_yb
                  sl = pybank[:, (m // 2) * 64 : (m // 2 + 1) * 64]
                  for ki, xk in ((0, x0), (1, x1), (2, x2)):
                      nc.tensor.matmul(
                          sl,
                          wT_s[:, ki * OUTC + m * 128 : ki * OUTC + (m + 1) * 128],
                          xk[:],
                          start=(ki == 0),
                          stop=(ki == 2),
                      )
                  nc.scalar.activation(
                      yscr[:],
                      sl,
                      mybir.ActivationFunctionType.Relu,
                      bias=bias8_s[:, m : m + 1],
                      scale=1.0,
                      accum_out=stile[:, m : m + 1],
                  )

              for m in range(8):
                  nc.tensor.matmul(
                      psum_l[:],
                      stile[:, m : m + 1],
                      hw8_s[:, m * NCLS : (m + 1) * NCLS],
                      start=(m == 0),
                      stop=False,
                  )
              # head_b folded in as a rank-1 (K=1) accumulation: ones.T @ hb
              nc.tensor.matmul(
                  psum_l[:], ones1[:], hb_s[:], start=False, stop=True
              )
              nc.scalar.copy(outs[:], psum_l[:])
              nc.gpsimd.dma_start(out=out[:], in_=outs[:])

    if split:
        _split_multiwaits(nc)
    return nc


_NC = None


def _get_nc():
    global _NC
    if _NC is None:
        _NC = _build_nc()
    return _NC


def make_in_maps(
    data0, data1, data2, conv_w, bn_gamma, bn_beta, bn_mean, bn_var, head_w, head_b
):
    import ml_dtypes

    f = np.float32
    bf = ml_dtypes.bfloat16
    data0 = np.ascontiguousarray(np.asarray(data0, dtype=f), dtype=bf)
    data1 = np.ascontiguousarray(np.asarray(data1, dtype=f), dtype=bf)
    data2 = np.ascontiguousarray(np.asarray(data2, dtype=f), dtype=bf)
    conv_w = np.asarray(conv_w, dtype=f)
    bn_gamma = np.asarray(bn_gamma, dtype=f)
    bn_beta = np.asarray(bn_beta, dtype=f)
    bn_mean = np.asarray(bn_mean, dtype=f)
    bn_var = np.asarray(bn_var, dtype=f)
    head_w = np.asarray(head_w, dtype=f)
    head_b = np.asarray(head_b, dtype=f)

    inv = (bn_gamma / np.sqrt(bn_var + np.float32(1e-5))).astype(f)
    w_folded = (conv_w * inv[None, :]).astype(f)  # [384, 1024]
    b_folded = (bn_beta - bn_mean * inv).astype(f)  # [1024]

    # wT[p, k*1024+j] = W'[k*128+p, j]  (K-chunk-major along free dim)
    wT = np.ascontiguousarray(
        w_folded.reshape(3, 128, OUTC).transpose(1, 0, 2).reshape(128, 3 * OUTC),
        dtype=bf,
    )
    # bias8[p, m] = b'[m*128+p]
    bias8 = np.ascontiguousarray(b_folded.reshape(8, 128).T)
    # hw8[p, m*40+q] = head_w[m*128+p, q] / 64   (1/64 folds the mean-pool)
    hw8 = np.ascontiguousarray(
        (head_w / np.float32(N2)).reshape(8, 128, NCLS).transpose(1, 0, 2).reshape(128, 8 * NCLS)
    )
    hb = np.ascontiguousarray(head_b.reshape(1, NCLS))

    in_maps = []
    for c in range(N_CORES):
        in_maps.append(
            {
                "d0": data0[c * D0 : (c + 1) * D0],
                "d1": data1[c * D1 : (c + 1) * D1],
                "d2": data2[c * D2 : (c + 1) * D2],
                "wT": wT,
                "bias8": bias8,
                "hw8": hw8,
                "hb": hb,
            }
        )
    return in_maps


_RUNNER = None


def _make_runner(nc):
    """Jitted SPMD executor (mirrors bass2jax.run_bass_via_pjrt but reuses
    one jit so repeated calls don't re-trace/re-compile)."""
    if True:
        import jax
        from jax.experimental.shard_map import shard_map
        from jax.sharding import Mesh, PartitionSpec

        from concourse import bass2jax, mybir as mb

        bass2jax.install_neuronx_cc_hook()
        partition_name = (
            nc.partition_id_tensor.name if nc.partition_id_tensor else None
        )
        in_names, out_names, out_avals, zero_outs = [], [], [], []
        for alloc in nc.m.functions[0].allocations:
            if not isinstance(alloc, mb.MemoryLocationSet):
                continue
            name = alloc.memorylocations[0].name
            if alloc.kind == "ExternalInput":
                if name != partition_name:
                    in_names.append(name)
            elif alloc.kind == "ExternalOutput":
                out_names.append(name)
                shape = tuple(alloc.tensor_shape)
                dtype = mb.dt.np(alloc.dtype)
                out_avals.append(jax.core.ShapedArray(shape, dtype))
                zero_outs.append(np.zeros(shape, dtype))
        n_params = len(in_names)
        all_in_names = in_names + out_names
        if partition_name is not None:
            all_in_names = all_in_names + [partition_name]

        def _body(*args):
            operands = list(args)
            if partition_name is not None:
                operands.append(bass2jax.partition_id_tensor())
            outs = bass2jax._bass_exec_p.bind(
                *operands,
                out_avals=tuple(out_avals),
                in_names=tuple(all_in_names),
                out_names=tuple(out_names),
                lowering_input_output_aliases=(),
                sim_require_finite=True,
                sim_require_nnan=True,
                nc=nc,
            )
            return tuple(outs)

        devices = jax.devices()[:N_CORES]
        mesh = Mesh(np.asarray(devices), ("core",))
        n_outs = len(out_avals)
        in_specs = (PartitionSpec("core"),) * (n_params + n_outs)
        out_specs = (PartitionSpec("core"),) * n_outs
        # No donation: the kernel writes every element of "out", so the
        # zero placeholder inputs can live on device and be reused.
        sharded = jax.jit(
            shard_map(
                _body,
                mesh=mesh,
                in_specs=in_specs,
                out_specs=out_specs,
                check_rep=False,
            ),
            keep_unused=True,
        )
        return dict(
            nc=nc,
            sharded=sharded,
            in_names=in_names,
            out_names=out_names,
            out_avals=out_avals,
            zero_outs=zero_outs,
            mesh=mesh,
        )


def _get_runner():
    global _RUNNER
    if _RUNNER is None:
        _RUNNER = _make_runner(_get_nc())
    return _RUNNER


def _concat_inputs(r, in_maps):
    return [
        np.concatenate([np.asarray(m[name]) for m in in_maps], axis=0)
        for name in r["in_names"]
    ]


def _concat_zeros(r):
    return [
        np.zeros((N_CORES * z.shape[0], *z.shape[1:]), z.dtype)
        for z in r["zero_outs"]
    ]


def _run(r, concat_in, concat_zeros=None):
    if concat_zeros is None:
        concat_zeros = _concat_zeros(r)
    out_arrs = r["sharded"](*concat_in, *concat_zeros)
    return out_arrs


def kernel(**inputs) -> np.ndarray:
    r = _get_runner()
    in_maps = make_in_maps(**inputs)
    out_arrs = _run(r, _concat_inputs(r, in_maps))
    return np.asarray(out_arrs[r["out_names"].index("out")])


def device_place_and_time(r, inputs, iters=20, batches=4):
    """Pre-place inputs on device, then time batches of back-to-back
    dispatches.  Returns (per-call seconds list, out array)."""
    import time

    import jax
    from jax.sharding import NamedSharding, PartitionSpec

    sharding = NamedSharding(r["mesh"], PartitionSpec("core"))
    concat_in = _concat_inputs(r, make_in_maps(**inputs))
    dev_in = [jax.device_put(a, sharding) for a in concat_in]
    dev_zeros = [jax.device_put(z, sharding) for z in _concat_zeros(r)]
    out_arrs = _run(r, dev_in, dev_zeros)  # warm
    out = np.asarray(out_arrs[r["out_names"].index("out")])
    times = []
    for _ in range(batches):
        t0 = time.perf_counter()
        last = None
        for _ in range(iters):
            last = _run(r, dev_in, dev_zeros)
        jax.block_until_ready(last)
        t1 = time.perf_counter()
        times.append((t1 - t0) / iters)
    return times, out



# revision 20
# speedup vs baseline: 2.2715x; 1.1557x over previous
"""Trainium2 Bass kernel for nn_ClsHeader (octree pooling classifier head).

Data-parallel over the batch dimension: each of the 8 NeuronCores processes
one sample (its full octree subtree), weights are replicated, outputs are
gathered host-side.  No collectives needed.

Per-core pipeline:
  - data0 [32768,128] (depth 5): four 4MB DMA loads (alternating between the
    SP and ACT HWDGE rings) with partition = depth-3 subtree (64 contiguous
    rows = 32KB per partition); per-partition max-pool as an IN-PLACE
    contiguous tensor-tensor max tree (4096->...->128, 6 DVE ops; ~0.9
    ns/elem vs ~1.5 ns/elem for the strided reduce) -> [128 d3-nodes,
    128 ch]; PE transposes collect all four tiles into one PSUM bank; one
    final contiguous 8->1 reduce -> x0 [128 ch, 64 d2-nodes].
  - data1 [4096,128] (depth 4): partition = d3 node (8 rows); 3-round
    in-place TT max tree over the sibling axis, 4 PE transposes, 8->1 reduce.
  - data2 [512,128] (depth 3): partition = d2 node (8 rows), 3-round tree.
  - Conv1x1+BN folded host-side into W' = conv_w*inv, b' = beta-mean*inv.
    y^T = W'^T @ x^T as 24 accumulating matmuls (3 K-chunks x 8 M-chunks)
    into two PSUM banks (groups sequential per bank; ScalarE drains one bank
    while PE fills the other).
  - ScalarE activation(Relu, bias=b', accum_out) fuses bias+relu+node-sum.
  - Head matmul: logit[1,40] = sum_m s_m^T @ (head_w/64)_m + head_b; the
    output DMA goes via GPSIMD so the SP ring never blocks on compute.

The walrus build here accepts only one sync-wait per instruction, so
_split_multiwaits() rewrites the scheduled program, moving extra waits onto
single-wait NOPs.  kernel() runs through a cached jitted shard_map executor
(the same custom-call path run_bass_kernel_spmd uses under axon) so repeated
calls do not re-trace or re-compile.

Precision: the three data tensors, the pooling tree, the transposes and the
conv weights run in bf16 (host converts; rel-err vs the fp32 reference is
~1.4e-3, well inside the 2e-2 gate).  This halves the dominant HBM traffic
(18.25 MiB -> 9.13 MiB per core) AND doubles DVE pooling throughput: DVE
tensor_tensor hits the 2x_1P perf mode only for 2-byte dtypes with step-1
access (cost ~ 58 + FD/2 cycles @ 0.96 GHz), so the bf16 max tree runs
~0.52 ns/elem vs ~1.04 fp32.  Per-rep engine budgets: DMA ~25us (9.57 MB
at the ~380 GB/s per-core aggregate cap), DVE ~23us, PE/ACT tiny -- the
kernel stays DMA-bound at the (halved) memory roofline.  uint8/fp8 would
halve DMA again but DVE has no 8-bit packing (1x mode, ~38us of pooling),
so 16-bit is the sweet spot.  gpsimd cannot help pool (walrus rejects
TensorTensor on Pool; it is a software DSP) and ACT has no max op, so all
pooling is DVE.  BN is folded into the conv weights host-side; the conv
accumulates in fp32 PSUM and bias/relu/node-sum/head stay fp32.

Measured fp32 baseline (repeat-slope, drift-cancelled): loads-only
~49.5us/rep, compute-only ~35us/rep, full ~50.5us/rep (DMA-bound).
"""

import os
import sys

for _p in ("/opt/trn_rl_repo", "/root/.axon_site/_ro/trn_rl_repo"):
    if os.path.isdir(_p) and _p not in sys.path:
        sys.path.append(_p)

import numpy as np

import concourse.bass as bass
import concourse.mybir as mybir
import concourse.tile as tile
from concourse.bass_utils import run_bass_kernel_spmd
from concourse.masks import make_identity

F32 = mybir.dt.float32
BF16 = mybir.dt.bfloat16
FP8 = mybir.dt.float8e4
N_CORES = 8
# How many of data0's four [8192,128] tiles stream as fp8e4m3 instead of
# bf16.  fp8 halves those tiles' DMA bytes; the cost is the first max-pool
# level running in the DVE's 1x mode (8-bit operands get no 2x packing)
# while converting fp8->bf16; later levels run 2x on bf16.  Per fp8 tile:
# DMA -2.6us, DVE +2.1us, so shift tiles to fp8 until DVE ~= DMA.
F8_TILES = 2
D0, D1, D2 = 32768, 4096, 512  # per-core (per-sample) rows at depths 5/4/3
C = 128  # channels per input level
OUTC = 1024  # conv output channels
NCLS = 40
N2 = 64  # depth-2 nodes per sample
AX = mybir.AxisListType.X


def _split_multiwaits(nc):
    """The walrus build in this container accepts only ONE sync-wait per
    instruction; move extra waits onto dedicated NOPs inserted just before
    the owning instruction (same engine, so sequencer order is preserved)."""
    n_split = 0
    for f in nc.m.functions:
        for bb in f.blocks:
            out = []
            changed = False
            for inst in bb.instructions:
                si = inst.sync_info
                waits = list(si.on_wait) if si is not None else []
                if len(waits) > 1:
                    for j, w in enumerate(waits[:-1]):
                        nop = mybir.InstNoOp(
                            name=f"{inst.name}-wsplit{j}", ins=[], outs=[]
                        )
                        nop.engine = inst.engine
                        nop.sync_info = mybir.SyncInfo(on_wait=[w], on_update=[])
                        out.append(nop)
                    si.on_wait = [waits[-1]]
                    changed = True
                    n_split += 1
                out.append(inst)
            if changed:
                bb.instructions = out
    return n_split


def _build_nc(split=True, repeat=1, mode="full", d0_split="all", ld_bufs=5,
              f8_tiles=None):
    # mode: "full" | "dma" (loads only, no compute) | "compute" (no big loads)
    # d0_split: "all" = every d0 tile halved across both rings;
    #           "first" = only tile 0 halved, others alternate rings.
    f8 = F8_TILES if f8_tiles is None else f8_tiles
    nc = bass.Bass("TRN2", num_devices=N_CORES)
    d0a = (
        nc.dram_tensor("d0a", [f8 * 8192, C], FP8, kind="ExternalInput")
        if f8 > 0
        else None
    )
    d0b = (
        nc.dram_tensor("d0b", [(4 - f8) * 8192, C], BF16, kind="ExternalInput")
        if f8 < 4
        else None
    )
    d1 = nc.dram_tensor("d1", [D1, C], BF16, kind="ExternalInput")
    d2 = nc.dram_tensor("d2", [D2, C], BF16, kind="ExternalInput")
    wT = nc.dram_tensor("wT", [128, 3 * OUTC], BF16, kind="ExternalInput")
    bias8 = nc.dram_tensor("bias8", [128, 8], F32, kind="ExternalInput")
    hw8 = nc.dram_tensor("hw8", [128, 8 * NCLS], F32, kind="ExternalInput")
    hb = nc.dram_tensor("hb", [1, NCLS], F32, kind="ExternalInput")
    out = nc.dram_tensor("out", [1, NCLS], F32, kind="ExternalOutput")

    with tile.TileContext(nc) as tc:
        with (
            tc.tile_pool(name="consts", bufs=1) as consts,
            tc.tile_pool(name="inp", bufs=4) as inp,
            tc.tile_pool(name="work", bufs=1) as work,
            tc.tile_pool(name="pt", bufs=4, space="PSUM") as pt,
            tc.tile_pool(name="py", bufs=1, space="PSUM") as py,
        ):
            wT_s = consts.tile([128, 3 * OUTC], BF16)
            nc.scalar.dma_start(out=wT_s[:], in_=wT[:])
            bias8_s = consts.tile([128, 8], F32)
            nc.scalar.dma_start(out=bias8_s[:], in_=bias8[:])
            hw8_s = consts.tile([128, 8 * NCLS], F32)
            nc.scalar.dma_start(out=hw8_s[:], in_=hw8[:])
            hb_s = consts.tile([1, NCLS], F32)
            nc.scalar.dma_start(out=hb_s[:], in_=hb[:])
            ident = consts.tile([128, 128], BF16)
            make_identity(nc, ident[:])
            ones1 = consts.tile([1, 1], F32)
            nc.vector.memset(ones1[:], 1.0)
            # warm-up Relu so the ACT table DMA (~2.7us) overlaps the ramp
            # instead of sitting in front of the first real activation
            actwarm = consts.tile([1, 1], F32)
            nc.scalar.activation(
                actwarm[:], ones1[:], mybir.ActivationFunctionType.Relu
            )

            if mode == "compute":
                ld1c = consts.tile([128, 4, 8, C], BF16)
                nc.vector.memset(ld1c[:], 0.25)
                ld2c = consts.tile([64, 8, C], BF16)
                nc.vector.memset(ld2c[:], 0.25)
                ldcs = []
                for t in range(4):
                    ldc = consts.tile(
                        [128, 64, C], FP8 if t < f8 else BF16, name=f"ldc{t}"
                    )
                    nc.vector.memset(ldc[:], 0.25)
                    ldcs.append(ldc)

            for _rep in range(repeat):
              # bufs=2 so rep N+1's producers never stall behind rep N's
              # last consumer (a stalled ACT instruction would also block
              # the ACT-ring DMA configs queued behind it).
              x0 = work.tile([128, N2], BF16, tag="x0", bufs=2)
              x1 = work.tile([128, N2], BF16, tag="x1", bufs=2)
              x2 = work.tile([128, N2], BF16, tag="x2", bufs=2)
              stile = work.tile([128, 8], F32, tag="stile", bufs=2)
              yscr = work.tile([128, 64], F32, tag="yscr", bufs=2)
              outs = work.tile([1, NCLS], F32, tag="outs", bufs=2)

              # PSUM: one bank collects all four data0 transposes, one bank
              # the four data1 transposes, one the data2 transpose; plus two
              # banks for the conv output (so ScalarE drains one while PE
              # fills the other) and one for the head logits = 6 of 8 banks.
              tpb0 = pt.tile([128, 512], BF16, tag="tpb0", bufs=1)
              tpb1 = pt.tile([128, 512], BF16, tag="tpb1", bufs=1)
              psum_ya = py.tile([128, 256], F32, tag="pya")
              psum_yb = py.tile([128, 256], F32, tag="pyb")
              psum_l = py.tile([1, NCLS], F32, tag="pl")

              # ---- data0: partition = d3 subtree (64 contiguous rows) ----
              # Every tile is split across the SP and ACT HWDGE rings so
              # tiles complete in order at an even cadence (keeps the
              # DVE tree pipeline fed and both rings balanced end-to-end).
              # Tiles 0..f8-1 stream as fp8e4m3, the rest as bf16.
              d0av = (
                  d0a[:].rearrange("(t p k) c -> t p k c", t=f8, p=128, k=64)
                  if f8 > 0
                  else None
              )
              d0bv = (
                  d0b[:].rearrange("(t p k) c -> t p k c", t=4 - f8, p=128, k=64)
                  if f8 < 4
                  else None
              )
              ld0s = []
              for t in range(4):
                  is8 = t < f8
                  if mode == "compute":
                      ld = ldcs[t]
                  else:
                      ld = inp.tile(
                          [128, 64, C],
                          FP8 if is8 else BF16,
                          bufs=ld_bufs,
                          tag="ld8" if is8 else "ld",
                      )
                      dv = d0av[t] if is8 else d0bv[t - f8]
                      if d0_split == "all" or t == 0:
                          nc.sync.dma_start(
                              out=ld[:, 0:32, :], in_=dv[:, 0:32, :]
                          )
                          nc.scalar.dma_start(
                              out=ld[:, 32:64, :], in_=dv[:, 32:64, :]
                          )
                      else:
                          eng = nc.sync if t % 2 == 0 else nc.scalar
                          eng.dma_start(out=ld[:], in_=dv)
                  ld0s.append(ld)

              # ---- data1 / data2 loads ----
              ld1 = ld1c if mode == "compute" else inp.tile(
                  [128, 4, 8, C], BF16, bufs=1, tag="ld1"
              )
              ld2 = ld2c if mode == "compute" else inp.tile(
                  [64, 8, C], BF16, bufs=1, tag="ld2"
              )
              if mode != "compute":
                  d1v = d1[:].rearrange("(j p k) c -> p j k c", j=4, p=128, k=8)
                  nc.sync.dma_start(out=ld1[:, 0:2], in_=d1v[:, 0:2])
                  nc.scalar.dma_start(out=ld1[:, 2:4], in_=d1v[:, 2:4])
                  nc.scalar.dma_start(
                      out=ld2[:], in_=d2[:].rearrange("(p k) c -> p k c", p=64, k=8)
                  )

              if mode == "dma":
                  dummy = work.tile([1, 8], F32, tag="dummy")
                  for t in range(4):
                      nc.vector.tensor_copy(
                          dummy[:, t : t + 1], ld0s[t][0:1, 0, 0:1]
                      )
                  nc.vector.tensor_copy(dummy[:, 4:5], ld1[0:1, 0, 0, 0:1])
                  nc.vector.tensor_copy(dummy[:, 5:6], ld2[0:1, 0, 0:1])
                  nc.vector.tensor_copy(outs[:, 0:8], dummy[:, 0:8])
                  nc.gpsimd.dma_start(out=out[:], in_=outs[:])
                  continue

              # stage-1 pool: in-place contiguous tensor-tensor max tree per
              # tile (64 sibling rows -> 1 per partition), then PE transpose
              # and a single contiguous 8:1 reduce over d3 siblings.  fp8
              # tiles run the first level as a converting fp8->bf16 max (1x
              # DVE mode) into a bf16 scratch; later levels run 2x on bf16.
              for t in range(4):
                  if t < f8:
                      s = work.tile([128, 32, C], BF16, tag="s8", bufs=2)
                      nc.vector.tensor_tensor(
                          out=s[:],
                          in0=ld0s[t][:, 0:32, :],
                          in1=ld0s[t][:, 32:64, :],
                          op=mybir.AluOpType.max,
                      )
                      src, n = s, 2048
                  else:
                      src, n = ld0s[t], 4096
                  ldf = src[:].rearrange("p k c -> p (k c)")
                  while n >= C:
                      nc.vector.tensor_tensor(
                          out=ldf[:, 0:n],
                          in0=ldf[:, 0:n],
                          in1=ldf[:, n : 2 * n],
                          op=mybir.AluOpType.max,
                      )
                      n //= 2
                  nc.tensor.transpose(
                      tpb0[:, t * 128 : (t + 1) * 128], src[:, 0, :], ident[:]
                  )
              nc.vector.reduce_max(
                  x0[:], tpb0[:].rearrange("p (g k) -> p g k", k=8), axis=AX
              )

              # ---- data1: partition = d3 node (8 contiguous rows) ----
              for sh in (4, 2, 1):
                  nc.vector.tensor_tensor(
                      out=ld1[:, :, 0:sh, :],
                      in0=ld1[:, :, 0:sh, :],
                      in1=ld1[:, :, sh : 2 * sh, :],
                      op=mybir.AluOpType.max,
                  )
              for j in range(4):
                  nc.tensor.transpose(
                      tpb1[:, j * 128 : (j + 1) * 128], ld1[:, j, 0, :], ident[:]
                  )
              nc.vector.reduce_max(
                  x1[:], tpb1[:].rearrange("p (g k) -> p g k", k=8), axis=AX
              )

              # ---- data2: partition = d2 node (8 contiguous rows) ----
              for sh in (4, 2, 1):
                  nc.vector.tensor_tensor(
                      out=ld2[:, 0:sh, :],
                      in0=ld2[:, 0:sh, :],
                      in1=ld2[:, sh : 2 * sh, :],
                      op=mybir.AluOpType.max,
                  )
              tp2 = pt.tile([128, 64], BF16, tag="tp", bufs=1)
              nc.tensor.transpose(tp2[:], ld2[:, 0, :], ident[0:64, 0:64])
              nc.scalar.copy(x2[:], tp2[:])

              if mode == "pool":
                  nc.vector.tensor_copy(outs[:, 0:1], x0[0:1, 0:1])
                  nc.vector.tensor_copy(outs[:, 1:2], x1[0:1, 0:1])
                  nc.vector.tensor_copy(outs[:, 2:3], x2[0:1, 0:1])
                  nc.gpsimd.dma_start(out=out[:], in_=outs[:])
                  continue

              # Conv matmuls: 8 m-chunks x 3 K-chunks; accumulation groups
              # run back-to-back within each bank.  ScalarE activation
              # (bias + relu + node-sum via accum_out) drains each slice.
              for m in range(8):
                  # parity-interleaved banks: ScalarE drains slice m in one
                  # bank while PE runs group m+1's matmuls in the other
                  pybank = psum_ya if m % 2 == 0 else psum_yb
                  sl = pybank[:, (m // 2) * 64 : (m // 2 + 1) * 64]
                  for ki, xk in ((0, x0), (1, x1), (2, x2)):
                      nc.tensor.matmul(
                          sl,
                          wT_s[:, ki * OUTC + m * 128 : ki * OUTC + (m + 1) * 128],
                          xk[:],
                          start=(ki == 0),
                          stop=(ki == 2),
                      )
                  nc.scalar.activation(
                      yscr[:],
                      sl,
                      mybir.ActivationFunctionType.Relu,
                      bias=bias8_s[:, m : m + 1],
                      scale=1.0,
                      accum_out=stile[:, m : m + 1],
                  )

              for m in range(8):
                  nc.tensor.matmul(
                      psum_l[:],
                      stile[:, m : m + 1],
                      hw8_s[:, m * NCLS : (m + 1) * NCLS],
                      start=(m == 0),
                      stop=False,
                  )
              # head_b folded in as a rank-1 (K=1) accumulation: ones.T @ hb
              nc.tensor.matmul(
                  psum_l[:], ones1[:], hb_s[:], start=False, stop=True
              )
              nc.scalar.copy(outs[:], psum_l[:])
              nc.gpsimd.dma_start(out=out[:], in_=outs[:])

    if split:
        _split_multiwaits(nc)
    return nc


_NC = None


def _get_nc():
    global _NC
    if _NC is None:
        _NC = _build_nc()
    return _NC


def make_in_maps(
    data0, data1, data2, conv_w, bn_gamma, bn_beta, bn_mean, bn_var, head_w, head_b
):
    import ml_dtypes

    f = np.float32
    bf = ml_dtypes.bfloat16
    f8 = ml_dtypes.float8_e4m3
    rows8 = F8_TILES * 8192  # per-core data0 rows streamed as fp8
    d0c = np.asarray(data0, dtype=f).reshape(N_CORES, D0, C)
    data0a = np.ascontiguousarray(d0c[:, :rows8], dtype=f8)
    data0b = np.ascontiguousarray(d0c[:, rows8:], dtype=bf)
    data1 = np.ascontiguousarray(np.asarray(data1, dtype=f), dtype=bf)
    data2 = np.ascontiguousarray(np.asarray(data2, dtype=f), dtype=bf)
    conv_w = np.asarray(conv_w, dtype=f)
    bn_gamma = np.asarray(bn_gamma, dtype=f)
    bn_beta = np.asarray(bn_beta, dtype=f)
    bn_mean = np.asarray(bn_mean, dtype=f)
    bn_var = np.asarray(bn_var, dtype=f)
    head_w = np.asarray(head_w, dtype=f)
    head_b = np.asarray(head_b, dtype=f)

    inv = (bn_gamma / np.sqrt(bn_var + np.float32(1e-5))).astype(f)
    w_folded = (conv_w * inv[None, :]).astype(f)  # [384, 1024]
    b_folded = (bn_beta - bn_mean * inv).astype(f)  # [1024]

    # wT[p, k*1024+j] = W'[k*128+p, j]  (K-chunk-major along free dim)
    wT = np.ascontiguousarray(
        w_folded.reshape(3, 128, OUTC).transpose(1, 0, 2).reshape(128, 3 * OUTC),
        dtype=bf,
    )
    # bias8[p, m] = b'[m*128+p]
    bias8 = np.ascontiguousarray(b_folded.reshape(8, 128).T)
    # hw8[p, m*40+q] = head_w[m*128+p, q] / 64   (1/64 folds the mean-pool)
    hw8 = np.ascontiguousarray(
        (head_w / np.float32(N2)).reshape(8, 128, NCLS).transpose(1, 0, 2).reshape(128, 8 * NCLS)
    )
    hb = np.ascontiguousarray(head_b.reshape(1, NCLS))

    in_maps = []
    for c in range(N_CORES):
        m = {
            "d1": data1[c * D1 : (c + 1) * D1],
            "d2": data2[c * D2 : (c + 1) * D2],
            "wT": wT,
            "bias8": bias8,
            "hw8": hw8,
            "hb": hb,
        }
        if F8_TILES > 0:
            m["d0a"] = data0a[c]
        if F8_TILES < 4:
            m["d0b"] = data0b[c]
        in_maps.append(m)
    return in_maps


_RUNNER = None


def _make_runner(nc):
    """Jitted SPMD executor (mirrors bass2jax.run_bass_via_pjrt but reuses
    one jit so repeated calls don't re-trace/re-compile)."""
    if True:
        import jax
        from jax.experimental.shard_map import shard_map
        from jax.sharding import Mesh, PartitionSpec

        from concourse import bass2jax, mybir as mb

        bass2jax.install_neuronx_cc_hook()
        partition_name = (
            nc.partition_id_tensor.name if nc.partition_id_tensor else None
        )
        in_names, out_names, out_avals, zero_outs = [], [], [], []
        for alloc in nc.m.functions[0].allocations:
            if not isinstance(alloc, mb.MemoryLocationSet):
                continue
            name = alloc.memorylocations[0].name
            if alloc.kind == "ExternalInput":
                if name != partition_name:
                    in_names.append(name)
            elif alloc.kind == "ExternalOutput":
                out_names.append(name)
                shape = tuple(alloc.tensor_shape)
                dtype = mb.dt.np(alloc.dtype)
                out_avals.append(jax.core.ShapedArray(shape, dtype))
                zero_outs.append(np.zeros(shape, dtype))
        n_params = len(in_names)
        all_in_names = in_names + out_names
        if partition_name is not None:
            all_in_names = all_in_names + [partition_name]

        def _body(*args):
            operands = list(args)
            if partition_name is not None:
                operands.append(bass2jax.partition_id_tensor())
            outs = bass2jax._bass_exec_p.bind(
                *operands,
                out_avals=tuple(out_avals),
                in_names=tuple(all_in_names),
                out_names=tuple(out_names),
                lowering_input_output_aliases=(),
                sim_require_finite=True,
                sim_require_nnan=True,
                nc=nc,
            )
            return tuple(outs)

        devices = jax.devices()[:N_CORES]
        mesh = Mesh(np.asarray(devices), ("core",))
        n_outs = len(out_avals)
        in_specs = (PartitionSpec("core"),) * (n_params + n_outs)
        out_specs = (PartitionSpec("core"),) * n_outs
        # No donation: the kernel writes every element of "out", so the
        # zero placeholder inputs can live on device and be reused.
        sharded = jax.jit(
            shard_map(
                _body,
                mesh=mesh,
                in_specs=in_specs,
                out_specs=out_specs,
                check_rep=False,
            ),
            keep_unused=True,
        )
        return dict(
            nc=nc,
            sharded=sharded,
            in_names=in_names,
            out_names=out_names,
            out_avals=out_avals,
            zero_outs=zero_outs,
            mesh=mesh,
        )


def _get_runner():
    global _RUNNER
    if _RUNNER is None:
        _RUNNER = _make_runner(_get_nc())
    return _RUNNER


def _concat_inputs(r, in_maps):
    return [
        np.concatenate([np.asarray(m[name]) for m in in_maps], axis=0)
        for name in r["in_names"]
    ]


def _concat_zeros(r):
    return [
        np.zeros((N_CORES * z.shape[0], *z.shape[1:]), z.dtype)
        for z in r["zero_outs"]
    ]


def _run(r, concat_in, concat_zeros=None):
    if concat_zeros is None:
        concat_zeros = _concat_zeros(r)
    out_arrs = r["sharded"](*concat_in, *concat_zeros)
    return out_arrs


def kernel(**inputs) -> np.ndarray:
    r = _get_runner()
    in_maps = make_in_maps(**inputs)
    out_arrs = _run(r, _concat_inputs(r, in_maps))
    return np.asarray(out_arrs[r["out_names"].index("out")])


def device_place_and_time(r, inputs, iters=20, batches=4):
    """Pre-place inputs on device, then time batches of back-to-back
    dispatches.  Returns (per-call seconds list, out array)."""
    import time

    import jax
    from jax.sharding import NamedSharding, PartitionSpec

    sharding = NamedSharding(r["mesh"], PartitionSpec("core"))
    concat_in = _concat_inputs(r, make_in_maps(**inputs))
    dev_in = [jax.device_put(a, sharding) for a in concat_in]
    dev_zeros = [jax.device_put(z, sharding) for z in _concat_zeros(r)]
    out_arrs = _run(r, dev_in, dev_zeros)  # warm
    out = np.asarray(out_arrs[r["out_names"].index("out")])
    times = []
    for _ in range(batches):
        t0 = time.perf_counter()
        last = None
        for _ in range(iters):
            last = _run(r, dev_in, dev_zeros)
        jax.block_until_ready(last)
        t1 = time.perf_counter()
        times.append((t1 - t0) / iters)
    return times, out



# revision 30
# speedup vs baseline: 2.9195x; 1.2853x over previous
"""Trainium2 Bass kernel for nn_ClsHeader (octree pooling classifier head).

Data-parallel over the batch dimension: each of the 8 NeuronCores processes
one sample (its full octree subtree), weights are replicated, outputs are
gathered host-side.  No collectives needed.

Per-core pipeline:
  - data0 [32768,128] (depth 5): four DMA tiles (2 fp8 + 2 bf16, halves
    split across two dma_start queues) with partition = depth-3 subtree
    (64 contiguous rows); fp8 tiles are upconverted to bf16 by the scalar
    engine; per-partition max-pool as an IN-PLACE contiguous tensor-tensor
    bf16 max tree (4096->...->128, 6 DVE ops) -> [128 d3-nodes, 128 ch];
    PE transposes collect all four tiles into one PSUM bank; one final
    contiguous 8->1 reduce -> x0 [128 ch, 64 d2-nodes].
  - data1 [4096,128] (depth 4): partition = d3 node (8 rows); 3-round
    in-place TT max tree over the sibling axis, 4 PE transposes, 8->1 reduce.
  - data2 [512,128] (depth 3): partition = d2 node (8 rows), 3-round tree.
  - Conv1x1+BN folded host-side into W' = conv_w*inv, b' = beta-mean*inv.
    y^T = W'^T @ x^T as 24 accumulating matmuls (3 K-chunks x 8 M-chunks)
    into two PSUM banks (groups sequential per bank; ScalarE drains one bank
    while PE fills the other).
  - ScalarE activation(Relu, bias=b', accum_out) fuses bias+relu+node-sum.
  - Head matmul: logit[1,40] = sum_m s_m^T @ (head_w/64)_m + head_b; the
    output DMA goes via GPSIMD so the SP ring never blocks on compute.

The walrus build here accepts only one sync-wait per instruction, so
_split_multiwaits() rewrites the scheduled program, moving extra waits onto
single-wait NOPs.  kernel() runs through a cached jitted shard_map executor
(the same custom-call path run_bass_kernel_spmd uses under axon) so repeated
calls do not re-trace or re-compile.

Precision (host converts; the 2e-2 rel-err gate leaves plenty of room):
  - data0 tiles 0-1 stream as fp8e4m3, tiles 2-3 and data1/data2 as bf16;
    the conv weights are bf16.  Measured rel-err vs the fp32 reference:
    3.1e-3.
  - The fp8 tiles are upconverted fp8->bf16 by the otherwise-idle scalar
    engine (ACT_TILES=2 full-tile Copy in two DMA-half chunks); the DVE
    then runs the identical all-bf16 max tree on every tile.
  - bf16 matters twice: it halves/quarters HBM bytes AND the DVE only has
    fast perf modes for 2-byte step-1 operands (measured ~4x packing for
    bf16 tensor_tensor on this silicon; 8-bit operands fall to 1x, which
    is why fp8 tiles are upconverted rather than pooled in fp8 -- a DVE-
    side converting first level costs ~3k extra DVE cycles/tile while the
    ACT copy is free parallelism).
Engine budgets per rep (measured by mode="dma"/"compute" repeat-slope):
DMA 7.5 MB/core ~15-20us depending on HBM contention (380-560 GB/s/core
observed across rounds; all 8 cores share chip HBM), DVE ~13us, ACT ~13us,
PE ~1.5us.  The kernel is DMA-bound at the quantized memory roofline; at
fp32 it was ~50us, all-bf16 ~23.5us, this config ~16-20us.  More fp8
(F8_TILES=3..4) trades DMA for DVE/ACT time and loses once HBM is quiet.
Ring B's dma_start configs live on the SP queue with ring A (one HWDGE
ring sustains the same aggregate BW; configs on the ACT queue would stall
behind the ~3.5us Copy ops, and gpsimd SWDGE bulk loads wedge the device).
gpsimd cannot pool (TensorTensor fails the Pool-engine ISA check) and ACT
has no max op, so all pooling is DVE.  BN is folded into the conv weights
host-side; the conv accumulates in fp32 PSUM and bias/relu/node-sum/head
stay fp32.
"""

import os
import sys

for _p in ("/opt/trn_rl_repo", "/root/.axon_site/_ro/trn_rl_repo"):
    if os.path.isdir(_p) and _p not in sys.path:
        sys.path.append(_p)

import numpy as np

import concourse.bass as bass
import concourse.mybir as mybir
import concourse.tile as tile
from concourse.bass_utils import run_bass_kernel_spmd
from concourse.masks import make_identity

F32 = mybir.dt.float32
BF16 = mybir.dt.bfloat16
FP8 = mybir.dt.float8e4
N_CORES = 8
# How many of data0's four [8192,128] tiles stream as fp8e4m3 instead of
# bf16.  fp8 halves those tiles' DMA bytes; the cost is the first max-pool
# level running in the DVE's 1x mode (8-bit operands get no 2x packing)
# while converting fp8->bf16; later levels run 2x on bf16.  Per fp8 tile:
# DMA -2.6us, DVE +2.1us, so shift tiles to fp8 until DVE ~= DMA.
F8_TILES = 2
# Of the F8_TILES fp8 tiles, the first ACT_TILES are instead upconverted
# fp8->bf16 by the (otherwise idle) scalar engine with a full-tile Copy,
# so the DVE runs the plain all-bf16 tree on them (offloads ~3k DVE cycles
# per tile onto ACT).
ACT_TILES = 2
# Which engine's queue carries ring-B DMA configs ("scalar"|"sync"|
# "gpsimd").  With ACT_TILES>0 the scalar engine runs ~3.5us Copy ops that
# would delay dma_start configs queued behind them, so ring B moves to the
# SP queue (a single HWDGE ring sustains the same aggregate BW; gpsimd
# SWDGE bulk loads wedge the device).
RINGB = "sync"
D0, D1, D2 = 32768, 4096, 512  # per-core (per-sample) rows at depths 5/4/3
C = 128  # channels per input level
OUTC = 1024  # conv output channels
NCLS = 40
N2 = 64  # depth-2 nodes per sample
AX = mybir.AxisListType.X


def _split_multiwaits(nc):
    """The walrus build in this container accepts only ONE sync-wait per
    instruction; move extra waits onto dedicated NOPs inserted just before
    the owning instruction (same engine, so sequencer order is preserved)."""
    n_split = 0
    for f in nc.m.functions:
        for bb in f.blocks:
            out = []
            changed = False
            for inst in bb.instructions:
                si = inst.sync_info
                waits = list(si.on_wait) if si is not None else []
                if len(waits) > 1:
                    for j, w in enumerate(waits[:-1]):
                        nop = mybir.InstNoOp(
                            name=f"{inst.name}-wsplit{j}", ins=[], outs=[]
                        )
                        nop.engine = inst.engine
                        nop.sync_info = mybir.SyncInfo(on_wait=[w], on_update=[])
                        out.append(nop)
                    si.on_wait = [waits[-1]]
                    changed = True
                    n_split += 1
                out.append(inst)
            if changed:
                bb.instructions = out
    return n_split


def _build_nc(split=True, repeat=1, mode="full", d0_split="all", ld_bufs=5,
              f8_tiles=None, act_tiles=None, ringb=None):
    # mode: "full" | "dma" (loads only, no compute) | "compute" (no big loads)
    # d0_split: "all" = every d0 tile halved across both rings;
    #           "first" = only tile 0 halved, others alternate rings.
    f8 = F8_TILES if f8_tiles is None else f8_tiles
    act = ACT_TILES if act_tiles is None else act_tiles
    ringb = RINGB if ringb is None else ringb
    assert 0 <= act <= f8 <= 4
    nc = bass.Bass("TRN2", num_devices=N_CORES)
    d0a = (
        nc.dram_tensor("d0a", [f8 * 8192, C], FP8, kind="ExternalInput")
        if f8 > 0
        else None
    )
    d0b = (
        nc.dram_tensor("d0b", [(4 - f8) * 8192, C], BF16, kind="ExternalInput")
        if f8 < 4
        else None
    )
    d1 = nc.dram_tensor("d1", [D1, C], BF16, kind="ExternalInput")
    d2 = nc.dram_tensor("d2", [D2, C], BF16, kind="ExternalInput")
    wT = nc.dram_tensor("wT", [128, 3 * OUTC], BF16, kind="ExternalInput")
    bias8 = nc.dram_tensor("bias8", [128, 8], F32, kind="ExternalInput")
    hw8 = nc.dram_tensor("hw8", [128, 8 * NCLS], F32, kind="ExternalInput")
    hb = nc.dram_tensor("hb", [1, NCLS], F32, kind="ExternalInput")
    out = nc.dram_tensor("out", [1, NCLS], F32, kind="ExternalOutput")

    with tile.TileContext(nc) as tc:
        ringb_eng = {
            "scalar": nc.scalar, "gpsimd": nc.gpsimd, "sync": nc.sync
        }[ringb]
        with (
            tc.tile_pool(name="consts", bufs=1) as consts,
            tc.tile_pool(name="inp", bufs=4) as inp,
            tc.tile_pool(name="work", bufs=1) as work,
            tc.tile_pool(name="pt", bufs=4, space="PSUM") as pt,
            tc.tile_pool(name="py", bufs=1, space="PSUM") as py,
        ):
            wT_s = consts.tile([128, 3 * OUTC], BF16)
            nc.scalar.dma_start(out=wT_s[:], in_=wT[:])
            bias8_s = consts.tile([128, 8], F32)
            nc.scalar.dma_start(out=bias8_s[:], in_=bias8[:])
            hw8_s = consts.tile([128, 8 * NCLS], F32)
            nc.scalar.dma_start(out=hw8_s[:], in_=hw8[:])
            hb_s = consts.tile([1, NCLS], F32)
            nc.scalar.dma_start(out=hb_s[:], in_=hb[:])
            ident = consts.tile([128, 128], BF16)
            make_identity(nc, ident[:])
            ones1 = consts.tile([1, 1], F32)
            nc.vector.memset(ones1[:], 1.0)
            # warm-up Relu so the ACT table DMA (~2.7us) overlaps the ramp
            # instead of sitting in front of the first real activation
            actwarm = consts.tile([1, 1], F32)
            nc.scalar.activation(
                actwarm[:], ones1[:], mybir.ActivationFunctionType.Relu
            )

            if mode == "compute":
                ld1c = consts.tile([128, 4, 8, C], BF16)
                nc.vector.memset(ld1c[:], 0.25)
                ld2c = consts.tile([64, 8, C], BF16)
                nc.vector.memset(ld2c[:], 0.25)
                ldcs = []
                for t in range(4):
                    ldc = consts.tile(
                        [128, 64, C], FP8 if t < f8 else BF16, name=f"ldc{t}"
                    )
                    nc.vector.memset(ldc[:], 0.25)
                    ldcs.append(ldc)

            for _rep in range(repeat):
              # bufs=2 so rep N+1's producers never stall behind rep N's
              # last consumer (a stalled ACT instruction would also block
              # the ACT-ring DMA configs queued behind it).
              x0 = work.tile([128, N2], BF16, tag="x0", bufs=2)
              x1 = work.tile([128, N2], BF16, tag="x1", bufs=2)
              x2 = work.tile([128, N2], BF16, tag="x2", bufs=2)
              stile = work.tile([128, 8], F32, tag="stile", bufs=2)
              yscr = work.tile([128, 64], F32, tag="yscr", bufs=2)
              outs = work.tile([1, NCLS], F32, tag="outs", bufs=2)

              # PSUM: one bank collects all four data0 transposes, one bank
              # the four data1 transposes, one the data2 transpose; plus two
              # banks for the conv output (so ScalarE drains one while PE
              # fills the other) and one for the head logits = 6 of 8 banks.
              tpb0 = pt.tile([128, 512], BF16, tag="tpb0", bufs=1)
              tpb1 = pt.tile([128, 512], BF16, tag="tpb1", bufs=1)
              psum_ya = py.tile([128, 256], F32, tag="pya")
              psum_yb = py.tile([128, 256], F32, tag="pyb")
              psum_l = py.tile([1, NCLS], F32, tag="pl")

              # ---- data0: partition = d3 subtree (64 contiguous rows) ----
              # Every tile is split across the SP and ACT HWDGE rings so
              # tiles complete in order at an even cadence (keeps the
              # DVE tree pipeline fed and both rings balanced end-to-end).
              # Tiles 0..f8-1 stream as fp8e4m3, the rest as bf16.
              d0av = (
                  d0a[:].rearrange("(t p k) c -> t p k c", t=f8, p=128, k=64)
                  if f8 > 0
                  else None
              )
              d0bv = (
                  d0b[:].rearrange("(t p k) c -> t p k c", t=4 - f8, p=128, k=64)
                  if f8 < 4
                  else None
              )
              ld0s = []
              for t in range(4):
                  is8 = t < f8
                  if mode == "compute":
                      ld = ldcs[t]
                  else:
                      ld = inp.tile(
                          [128, 64, C],
                          FP8 if is8 else BF16,
                          bufs=ld_bufs,
                          tag="ld8" if is8 else "ld",
                      )
                      dv = d0av[t] if is8 else d0bv[t - f8]
                      if d0_split == "all" or t == 0:
                          nc.sync.dma_start(
                              out=ld[:, 0:32, :], in_=dv[:, 0:32, :]
                          )
                          ringb_eng.dma_start(
                              out=ld[:, 32:64, :], in_=dv[:, 32:64, :]
                          )
                      else:
                          eng = nc.sync if t % 2 == 0 else ringb_eng
                          eng.dma_start(out=ld[:], in_=dv)
                  ld0s.append(ld)

              # ---- data1 / data2 loads ----
              ld1 = ld1c if mode == "compute" else inp.tile(
                  [128, 4, 8, C], BF16, bufs=1, tag="ld1"
              )
              ld2 = ld2c if mode == "compute" else inp.tile(
                  [64, 8, C], BF16, bufs=1, tag="ld2"
              )
              if mode != "compute":
                  d1v = d1[:].rearrange("(j p k) c -> p j k c", j=4, p=128, k=8)
                  nc.sync.dma_start(out=ld1[:, 0:2], in_=d1v[:, 0:2])
                  ringb_eng.dma_start(out=ld1[:, 2:4], in_=d1v[:, 2:4])
                  ringb_eng.dma_start(
                      out=ld2[:], in_=d2[:].rearrange("(p k) c -> p k c", p=64, k=8)
                  )

              if mode == "dma":
                  dummy = work.tile([1, 8], F32, tag="dummy")
                  for t in range(4):
                      nc.vector.tensor_copy(
                          dummy[:, t : t + 1], ld0s[t][0:1, 0, 0:1]
                      )
                  nc.vector.tensor_copy(dummy[:, 4:5], ld1[0:1, 0, 0, 0:1])
                  nc.vector.tensor_copy(dummy[:, 5:6], ld2[0:1, 0, 0:1])
                  nc.vector.tensor_copy(outs[:, 0:8], dummy[:, 0:8])
                  nc.gpsimd.dma_start(out=out[:], in_=outs[:])
                  continue

              # stage-1 pool: in-place contiguous tensor-tensor max tree per
              # tile (64 sibling rows -> 1 per partition), then PE transpose
              # and a single contiguous 8:1 reduce over d3 siblings.  fp8
              # tiles run the first level as a converting fp8->bf16 max (1x
              # DVE mode) into a bf16 scratch; later levels run 2x on bf16.
              for t in range(4):
                  if t < act:
                      # fp8 tile upconverted by the scalar engine (two half
                      # copies so each starts as soon as its DMA half lands);
                      # the DVE then runs the plain all-bf16 tree.
                      c8 = work.tile([128, 64, C], BF16, tag="c8", bufs=2)
                      nc.scalar.copy(c8[:, 0:32, :], ld0s[t][:, 0:32, :])
                      nc.scalar.copy(c8[:, 32:64, :], ld0s[t][:, 32:64, :])
                      src, n = c8, 4096
                  elif t < f8:
                      s = work.tile([128, 32, C], BF16, tag="s8", bufs=2)
                      nc.vector.tensor_tensor(
                          out=s[:],
                          in0=ld0s[t][:, 0:32, :],
                          in1=ld0s[t][:, 32:64, :],
                          op=mybir.AluOpType.max,
                      )
                      src, n = s, 2048
                  else:
                      src, n = ld0s[t], 4096
                  ldf = src[:].rearrange("p k c -> p (k c)")
                  while n >= C:
                      nc.vector.tensor_tensor(
                          out=ldf[:, 0:n],
                          in0=ldf[:, 0:n],
                          in1=ldf[:, n : 2 * n],
                          op=mybir.AluOpType.max,
                      )
                      n //= 2
                  nc.tensor.transpose(
                      tpb0[:, t * 128 : (t + 1) * 128], src[:, 0, :], ident[:]
                  )
              nc.vector.reduce_max(
                  x0[:], tpb0[:].rearrange("p (g k) -> p g k", k=8), axis=AX
              )

              # ---- data1: partition = d3 node (8 contiguous rows) ----
              for sh in (4, 2, 1):
                  nc.vector.tensor_tensor(
                      out=ld1[:, :, 0:sh, :],
                      in0=ld1[:, :, 0:sh, :],
                      in1=ld1[:, :, sh : 2 * sh, :],
                      op=mybir.AluOpType.max,
                  )
              for j in range(4):
                  nc.tensor.transpose(
                      tpb1[:, j * 128 : (j + 1) * 128], ld1[:, j, 0, :], ident[:]
                  )
              nc.vector.reduce_max(
                  x1[:], tpb1[:].rearrange("p (g k) -> p g k", k=8), axis=AX
              )

              # ---- data2: partition = d2 node (8 contiguous rows) ----
              for sh in (4, 2, 1):
                  nc.vector.tensor_tensor(
                      out=ld2[:, 0:sh, :],
                      in0=ld2[:, 0:sh, :],
                      in1=ld2[:, sh : 2 * sh, :],
                      op=mybir.AluOpType.max,
                  )
              tp2 = pt.tile([128, 64], BF16, tag="tp", bufs=1)
              nc.tensor.transpose(tp2[:], ld2[:, 0, :], ident[0:64, 0:64])
              nc.scalar.copy(x2[:], tp2[:])

              if mode == "pool":
                  nc.vector.tensor_copy(outs[:, 0:1], x0[0:1, 0:1])
                  nc.vector.tensor_copy(outs[:, 1:2], x1[0:1, 0:1])
                  nc.vector.tensor_copy(outs[:, 2:3], x2[0:1, 0:1])
                  nc.gpsimd.dma_start(out=out[:], in_=outs[:])
                  continue

              # Conv matmuls: 8 m-chunks x 3 K-chunks; accumulation groups
              # run back-to-back within each bank.  ScalarE activation
              # (bias + relu + node-sum via accum_out) drains each slice.
              for m in range(8):
                  # parity-interleaved banks: ScalarE drains slice m in one
                  # bank while PE runs group m+1's matmuls in the other
                  pybank = psum_ya if m % 2 == 0 else psum_yb
                  sl = pybank[:, (m // 2) * 64 : (m // 2 + 1) * 64]
                  for ki, xk in ((0, x0), (1, x1), (2, x2)):
                      nc.tensor.matmul(
                          sl,
                          wT_s[:, ki * OUTC + m * 128 : ki * OUTC + (m + 1) * 128],
                          xk[:],
                          start=(ki == 0),
                          stop=(ki == 2),
                      )
                  nc.scalar.activation(
                      yscr[:],
                      sl,
                      mybir.ActivationFunctionType.Relu,
                      bias=bias8_s[:, m : m + 1],
                      scale=1.0,
                      accum_out=stile[:, m : m + 1],
                  )

              for m in range(8):
                  nc.tensor.matmul(
                      psum_l[:],
                      stile[:, m : m + 1],
                      hw8_s[:, m * NCLS : (m + 1) * NCLS],
                      start=(m == 0),
                      stop=False,
                  )
              # head_b folded in as a rank-1 (K=1) accumulation: ones.T @ hb
              nc.tensor.matmul(
                  psum_l[:], ones1[:], hb_s[:], start=False, stop=True
              )
              nc.scalar.copy(outs[:], psum_l[:])
              nc.gpsimd.dma_start(out=out[:], in_=outs[:])

    if split:
        _split_multiwaits(nc)
    return nc


_NC = None


def _get_nc():
    global _NC
    if _NC is None:
        _NC = _build_nc()
    return _NC


def make_in_maps(
    data0, data1, data2, conv_w, bn_gamma, bn_beta, bn_mean, bn_var, head_w, head_b
):
    import ml_dtypes

    f = np.float32
    bf = ml_dtypes.bfloat16
    f8 = ml_dtypes.float8_e4m3
    rows8 = F8_TILES * 8192  # per-core data0 rows streamed as fp8
    d0c = np.asarray(data0, dtype=f).reshape(N_CORES, D0, C)
    data0a = np.ascontiguousarray(d0c[:, :rows8], dtype=f8)
    data0b = np.ascontiguousarray(d0c[:, rows8:], dtype=bf)
    data1 = np.ascontiguousarray(np.asarray(data1, dtype=f), dtype=bf)
    data2 = np.ascontiguousarray(np.asarray(data2, dtype=f), dtype=bf)
    conv_w = np.asarray(conv_w, dtype=f)
    bn_gamma = np.asarray(bn_gamma, dtype=f)
    bn_beta = np.asarray(bn_beta, dtype=f)
    bn_mean = np.asarray(bn_mean, dtype=f)
    bn_var = np.asarray(bn_var, dtype=f)
    head_w = np.asarray(head_w, dtype=f)
    head_b = np.asarray(head_b, dtype=f)

    inv = (bn_gamma / np.sqrt(bn_var + np.float32(1e-5))).astype(f)
    w_folded = (conv_w * inv[None, :]).astype(f)  # [384, 1024]
    b_folded = (bn_beta - bn_mean * inv).astype(f)  # [1024]

    # wT[p, k*1024+j] = W'[k*128+p, j]  (K-chunk-major along free dim)
    wT = np.ascontiguousarray(
        w_folded.reshape(3, 128, OUTC).transpose(1, 0, 2).reshape(128, 3 * OUTC),
        dtype=bf,
    )
    # bias8[p, m] = b'[m*128+p]
    bias8 = np.ascontiguousarray(b_folded.reshape(8, 128).T)
    # hw8[p, m*40+q] = head_w[m*128+p, q] / 64   (1/64 folds the mean-pool)
    hw8 = np.ascontiguousarray(
        (head_w / np.float32(N2)).reshape(8, 128, NCLS).transpose(1, 0, 2).reshape(128, 8 * NCLS)
    )
    hb = np.ascontiguousarray(head_b.reshape(1, NCLS))

    in_maps = []
    for c in range(N_CORES):
        m = {
            "d1": data1[c * D1 : (c + 1) * D1],
            "d2": data2[c * D2 : (c + 1) * D2],
            "wT": wT,
            "bias8": bias8,
            "hw8": hw8,
            "hb": hb,
        }
        if F8_TILES > 0:
            m["d0a"] = data0a[c]
        if F8_TILES < 4:
            m["d0b"] = data0b[c]
        in_maps.append(m)
    return in_maps


_RUNNER = None


def _make_runner(nc):
    """Jitted SPMD executor (mirrors bass2jax.run_bass_via_pjrt but reuses
    one jit so repeated calls don't re-trace/re-compile)."""
    if True:
        import jax
        from jax.experimental.shard_map import shard_map
        from jax.sharding import Mesh, PartitionSpec

        from concourse import bass2jax, mybir as mb

        bass2jax.install_neuronx_cc_hook()
        partition_name = (
            nc.partition_id_tensor.name if nc.partition_id_tensor else None
        )
        in_names, out_names, out_avals, zero_outs = [], [], [], []
        for alloc in nc.m.functions[0].allocations:
            if not isinstance(alloc, mb.MemoryLocationSet):
                continue
            name = alloc.memorylocations[0].name
            if alloc.kind == "ExternalInput":
                if name != partition_name:
                    in_names.append(name)
            elif alloc.kind == "ExternalOutput":
                out_names.append(name)
                shape = tuple(alloc.tensor_shape)
                dtype = mb.dt.np(alloc.dtype)
                out_avals.append(jax.core.ShapedArray(shape, dtype))
                zero_outs.append(np.zeros(shape, dtype))
        n_params = len(in_names)
        all_in_names = in_names + out_names
        if partition_name is not None:
            all_in_names = all_in_names + [partition_name]

        def _body(*args):
            operands = list(args)
            if partition_name is not None:
                operands.append(bass2jax.partition_id_tensor())
            outs = bass2jax._bass_exec_p.bind(
                *operands,
                out_avals=tuple(out_avals),
                in_names=tuple(all_in_names),
                out_names=tuple(out_names),
                lowering_input_output_aliases=(),
                sim_require_finite=True,
                sim_require_nnan=True,
                nc=nc,
            )
            return tuple(outs)

        devices = jax.devices()[:N_CORES]
        mesh = Mesh(np.asarray(devices), ("core",))
        n_outs = len(out_avals)
        in_specs = (PartitionSpec("core"),) * (n_params + n_outs)
        out_specs = (PartitionSpec("core"),) * n_outs
        # No donation: the kernel writes every element of "out", so the
        # zero placeholder inputs can live on device and be reused.
        sharded = jax.jit(
            shard_map(
                _body,
                mesh=mesh,
                in_specs=in_specs,
                out_specs=out_specs,
                check_rep=False,
            ),
            keep_unused=True,
        )
        return dict(
            nc=nc,
            sharded=sharded,
            in_names=in_names,
            out_names=out_names,
            out_avals=out_avals,
            zero_outs=zero_outs,
            mesh=mesh,
        )


def _get_runner():
    global _RUNNER
    if _RUNNER is None:
        _RUNNER = _make_runner(_get_nc())
    return _RUNNER


def _concat_inputs(r, in_maps):
    return [
        np.concatenate([np.asarray(m[name]) for m in in_maps], axis=0)
        for name in r["in_names"]
    ]


def _concat_zeros(r):
    return [
        np.zeros((N_CORES * z.shape[0], *z.shape[1:]), z.dtype)
        for z in r["zero_outs"]
    ]


def _run(r, concat_in, concat_zeros=None):
    if concat_zeros is None:
        concat_zeros = _concat_zeros(r)
    out_arrs = r["sharded"](*concat_in, *concat_zeros)
    return out_arrs


def kernel(**inputs) -> np.ndarray:
    r = _get_runner()
    in_maps = make_in_maps(**inputs)
    out_arrs = _run(r, _concat_inputs(r, in_maps))
    return np.asarray(out_arrs[r["out_names"].index("out")])


def device_place_and_time(r, inputs, iters=20, batches=4):
    """Pre-place inputs on device, then time batches of back-to-back
    dispatches.  Returns (per-call seconds list, out array)."""
    import time

    import jax
    from jax.sharding import NamedSharding, PartitionSpec

    sharding = NamedSharding(r["mesh"], PartitionSpec("core"))
    concat_in = _concat_inputs(r, make_in_maps(**inputs))
    dev_in = [jax.device_put(a, sharding) for a in concat_in]
    dev_zeros = [jax.device_put(z, sharding) for z in _concat_zeros(r)]
    out_arrs = _run(r, dev_in, dev_zeros)  # warm
    out = np.asarray(out_arrs[r["out_names"].index("out")])
    times = []
    for _ in range(batches):
        t0 = time.perf_counter()
        last = None
        for _ in range(iters):
            last = _run(r, dev_in, dev_zeros)
        jax.block_until_ready(last)
        t1 = time.perf_counter()
        times.append((t1 - t0) / iters)
    return times, out



# revision 40
# speedup vs baseline: 3.5747x; 1.2244x over previous
"""Trainium2 Bass kernel for nn_ClsHeader (octree pooling classifier head).

Data-parallel over the batch dimension: each of the 8 NeuronCores processes
one sample (its full octree subtree), weights are replicated, outputs are
gathered host-side.  No collectives needed.

Per-core pipeline:
  - data0 [32768,128] (depth 5): four DMA tiles (2 fp8 + 2 bf16, halves
    split across two dma_start queues) with partition = depth-3 subtree
    (64 contiguous rows); fp8 tiles are upconverted to bf16 by the scalar
    engine; per-partition max-pool as an IN-PLACE contiguous tensor-tensor
    bf16 max tree (4096->...->128, 6 DVE ops) -> [128 d3-nodes, 128 ch];
    PE transposes collect all four tiles into one PSUM bank; one final
    contiguous 8->1 reduce -> x0 [128 ch, 64 d2-nodes].
  - data1 [4096,128] (depth 4): partition = d3 node (8 rows); 3-round
    in-place TT max tree over the sibling axis, 4 PE transposes, 8->1 reduce.
  - data2 [512,128] (depth 3): partition = d2 node (8 rows), 3-round tree.
  - Conv1x1+BN folded host-side into W' = conv_w*inv, b' = beta-mean*inv.
    y^T = W'^T @ x^T as 24 accumulating matmuls (3 K-chunks x 8 M-chunks)
    into two PSUM banks (groups sequential per bank; ScalarE drains one bank
    while PE fills the other).
  - ScalarE activation(Relu, bias=b', accum_out) fuses bias+relu+node-sum.
  - Head matmul: logit[1,40] = sum_m s_m^T @ (head_w/64)_m + head_b; the
    output DMA goes via GPSIMD so the SP ring never blocks on compute.

The walrus build here accepts only one sync-wait per instruction, so
_split_multiwaits() rewrites the scheduled program, moving extra waits onto
single-wait NOPs.  kernel() runs through a cached jitted shard_map executor
(the same custom-call path run_bass_kernel_spmd uses under axon) so repeated
calls do not re-trace or re-compile.

Precision (host converts; the 2e-2 rel-err gate leaves plenty of room):
  - data0 tiles 0-1 stream as fp8e4m3, tiles 2-3 and data1/data2 as bf16;
    the conv weights are bf16.  Measured rel-err vs the fp32 reference:
    3.1e-3.
  - The fp8 tiles are upconverted fp8->bf16 by the otherwise-idle scalar
    engine (ACT_TILES=2 full-tile Copy in two DMA-half chunks); the DVE
    then runs the identical all-bf16 max tree on every tile.
  - bf16 matters twice: it halves/quarters HBM bytes AND the DVE only has
    fast perf modes for 2-byte step-1 operands (measured ~4x packing for
    bf16 tensor_tensor on this silicon; 8-bit operands fall to 1x, which
    is why fp8 tiles are upconverted rather than pooled in fp8 -- a DVE-
    side converting first level costs ~3k extra DVE cycles/tile while the
    ACT copy is free parallelism).
Engine budgets per rep (measured by mode="dma"/"compute" repeat-slope):
DMA 7.5 MB/core ~15-20us depending on HBM contention (380-560 GB/s/core
observed across rounds; all 8 cores share chip HBM), DVE ~13us, ACT ~13us,
PE ~1.5us.  The kernel is DMA-bound at the quantized memory roofline; at
fp32 it was ~50us, all-bf16 ~23.5us, this config ~16-20us.  Measured
dead ends: ACT saturates at 2 converted tiles (3:3/4:4 cost +4.5us per
extra tile end-to-end) but is not the binder (ACT_TILES=1 measures ~1us
worse); fp8 data1 with a DVE-converting first level (D1_MODE="dve") is
a wash (-0.55MB DMA vs +1.5k DVE cyc); DMA cannot cast dtypes (HWDGE
rejects fp8->bf16 loads; only gpsimd SWDGE casts, and bulk SWDGE wedges
the device); ring-B configs on the scalar queue cost +9us/rep (the Tile
scheduler does not hoist them ahead of the ACT Copies -- this is why
RINGB="sync" is load-bearing).
Ring B's dma_start configs live on the SP queue with ring A (one HWDGE
ring sustains the same aggregate BW; configs on the ACT queue would stall
behind the ~3.5us Copy ops, and gpsimd SWDGE bulk loads wedge the device).
gpsimd cannot pool (TensorTensor fails the Pool-engine ISA check) and ACT
has no max op, so all pooling is DVE.  BN is folded into the conv weights
host-side; the conv accumulates in fp32 PSUM and bias/relu/node-sum/head
stay fp32.
"""

import os
import sys

for _p in ("/opt/trn_rl_repo", "/root/.axon_site/_ro/trn_rl_repo"):
    if os.path.isdir(_p) and _p not in sys.path:
        sys.path.append(_p)

import numpy as np

import concourse.bass as bass
import concourse.mybir as mybir
import concourse.tile as tile
from concourse.bass_utils import run_bass_kernel_spmd
from concourse.masks import make_identity

F32 = mybir.dt.float32
BF16 = mybir.dt.bfloat16
FP8 = mybir.dt.float8e4
N_CORES = 8
# How many of data0's four [8192,128] tiles stream as fp8e4m3 instead of
# bf16.  fp8 halves those tiles' DMA bytes; the cost is the first max-pool
# level running in the DVE's 1x mode (8-bit operands get no 2x packing)
# while converting fp8->bf16; later levels run 2x on bf16.  Per fp8 tile:
# DMA -2.6us, DVE +2.1us, so shift tiles to fp8 until DVE ~= DMA.
F8_TILES = 2
# Of the F8_TILES fp8 tiles, the first ACT_TILES are instead upconverted
# fp8->bf16 by the (otherwise idle) scalar engine with a full-tile Copy,
# so the DVE runs the plain all-bf16 tree on them (offloads ~3k DVE cycles
# per tile onto ACT).
ACT_TILES = 2
# Which engine's queue carries ring-B DMA configs ("scalar"|"sync"|
# "gpsimd").  With ACT_TILES>0 the scalar engine runs ~3.5us Copy ops that
# would delay dma_start configs queued behind them, so ring B moves to the
# SP queue (a single HWDGE ring sustains the same aggregate BW; gpsimd
# SWDGE bulk loads wedge the device).
RINGB = "sync"
# data1 streaming dtype: "bf16" (default), or fp8 with the first tree level
# converting on the DVE ("dve", +1.5k DVE cyc) or a full upconvert Copy on
# the scalar engine ("act", +4.5k ACT cyc); either saves 0.55 MB of DMA.
D1_MODE = "bf16"
D0, D1, D2 = 32768, 4096, 512  # per-core (per-sample) rows at depths 5/4/3
C = 128  # channels per input level
OUTC = 1024  # conv output channels
NCLS = 40
N2 = 64  # depth-2 nodes per sample
AX = mybir.AxisListType.X


def _split_multiwaits(nc):
    """The walrus build in this container accepts only ONE sync-wait per
    instruction; move extra waits onto dedicated NOPs inserted just before
    the owning instruction (same engine, so sequencer order is preserved)."""
    n_split = 0
    for f in nc.m.functions:
        for bb in f.blocks:
            out = []
            changed = False
            for inst in bb.instructions:
                si = inst.sync_info
                waits = list(si.on_wait) if si is not None else []
                if len(waits) > 1:
                    for j, w in enumerate(waits[:-1]):
                        nop = mybir.InstNoOp(
                            name=f"{inst.name}-wsplit{j}", ins=[], outs=[]
                        )
                        nop.engine = inst.engine
                        nop.sync_info = mybir.SyncInfo(on_wait=[w], on_update=[])
                        out.append(nop)
                    si.on_wait = [waits[-1]]
                    changed = True
                    n_split += 1
                out.append(inst)
            if changed:
                bb.instructions = out
    return n_split


def _build_nc(split=True, repeat=1, mode="full", d0_split="all", ld_bufs=5,
              f8_tiles=None, act_tiles=None, ringb=None, d1_mode=None):
    # mode: "full" | "dma" (loads only, no compute) | "compute" (no big loads)
    # d0_split: "all" = every d0 tile halved across both rings;
    #           "first" = only tile 0 halved, others alternate rings.
    f8 = F8_TILES if f8_tiles is None else f8_tiles
    act = ACT_TILES if act_tiles is None else act_tiles
    ringb = RINGB if ringb is None else ringb
    d1_mode = D1_MODE if d1_mode is None else d1_mode
    assert 0 <= act <= f8 <= 4 and d1_mode in ("bf16", "dve", "act")
    nc = bass.Bass("TRN2", num_devices=N_CORES)
    d0a = (
        nc.dram_tensor("d0a", [f8 * 8192, C], FP8, kind="ExternalInput")
        if f8 > 0
        else None
    )
    d0b = (
        nc.dram_tensor("d0b", [(4 - f8) * 8192, C], BF16, kind="ExternalInput")
        if f8 < 4
        else None
    )
    d1 = nc.dram_tensor(
        "d1", [D1, C], BF16 if d1_mode == "bf16" else FP8, kind="ExternalInput"
    )
    d2 = nc.dram_tensor("d2", [D2, C], BF16, kind="ExternalInput")
    wT = nc.dram_tensor("wT", [128, 3 * OUTC], BF16, kind="ExternalInput")
    bias8 = nc.dram_tensor("bias8", [128, 8], F32, kind="ExternalInput")
    hw8 = nc.dram_tensor("hw8", [128, 8 * NCLS], F32, kind="ExternalInput")
    hb = nc.dram_tensor("hb", [1, NCLS], F32, kind="ExternalInput")
    out = nc.dram_tensor("out", [1, NCLS], F32, kind="ExternalOutput")

    with tile.TileContext(nc) as tc:
        ringb_eng = {
            "scalar": nc.scalar, "gpsimd": nc.gpsimd, "sync": nc.sync
        }[ringb]
        with (
            tc.tile_pool(name="consts", bufs=1) as consts,
            tc.tile_pool(name="inp", bufs=4) as inp,
            tc.tile_pool(name="work", bufs=1) as work,
            tc.tile_pool(name="pt", bufs=4, space="PSUM") as pt,
            tc.tile_pool(name="py", bufs=1, space="PSUM") as py,
        ):
            wT_s = consts.tile([128, 3 * OUTC], BF16)
            nc.scalar.dma_start(out=wT_s[:], in_=wT[:])
            bias8_s = consts.tile([128, 8], F32)
            nc.scalar.dma_start(out=bias8_s[:], in_=bias8[:])
            hw8_s = consts.tile([128, 8 * NCLS], F32)
            nc.scalar.dma_start(out=hw8_s[:], in_=hw8[:])
            hb_s = consts.tile([1, NCLS], F32)
            nc.scalar.dma_start(out=hb_s[:], in_=hb[:])
            ident = consts.tile([128, 128], BF16)
            make_identity(nc, ident[:])
            ones1 = consts.tile([1, 1], F32)
            nc.vector.memset(ones1[:], 1.0)
            # warm-up Relu so the ACT table DMA (~2.7us) overlaps the ramp
            # instead of sitting in front of the first real activation
            actwarm = consts.tile([1, 1], F32)
            nc.scalar.activation(
                actwarm[:], ones1[:], mybir.ActivationFunctionType.Relu
            )

            if mode == "compute":
                ld1c = consts.tile(
                    [128, 4, 8, C], BF16 if d1_mode == "bf16" else FP8
                )
                nc.vector.memset(ld1c[:], 0.25)
                ld2c = consts.tile([64, 8, C], BF16)
                nc.vector.memset(ld2c[:], 0.25)
                ldcs = []
                for t in range(4):
                    ldc = consts.tile(
                        [128, 64, C], FP8 if t < f8 else BF16, name=f"ldc{t}"
                    )
                    nc.vector.memset(ldc[:], 0.25)
                    ldcs.append(ldc)

            for _rep in range(repeat):
              # bufs=2 so rep N+1's producers never stall behind rep N's
              # last consumer (a stalled ACT instruction would also block
              # the ACT-ring DMA configs queued behind it).
              x0 = work.tile([128, N2], BF16, tag="x0", bufs=2)
              x1 = work.tile([128, N2], BF16, tag="x1", bufs=2)
              x2 = work.tile([128, N2], BF16, tag="x2", bufs=2)
              stile = work.tile([128, 8], F32, tag="stile", bufs=2)
              yscr = work.tile([128, 64], F32, tag="yscr", bufs=2)
              outs = work.tile([1, NCLS], F32, tag="outs", bufs=2)

              # PSUM: one bank collects all four data0 transposes, one bank
              # the four data1 transposes, one the data2 transpose; plus two
              # banks for the conv output (so ScalarE drains one while PE
              # fills the other) and one for the head logits = 6 of 8 banks.
              tpb0 = pt.tile([128, 512], BF16, tag="tpb0", bufs=1)
              tpb1 = pt.tile([128, 512], BF16, tag="tpb1", bufs=1)
              psum_ya = py.tile([128, 256], F32, tag="pya")
              psum_yb = py.tile([128, 256], F32, tag="pyb")
              psum_l = py.tile([1, NCLS], F32, tag="pl")

              # ---- data0: partition = d3 subtree (64 contiguous rows) ----
              # Every tile is split across the SP and ACT HWDGE rings so
              # tiles complete in order at an even cadence (keeps the
              # DVE tree pipeline fed and both rings balanced end-to-end).
              # Tiles 0..f8-1 stream as fp8e4m3, the rest as bf16.
              d0av = (
                  d0a[:].rearrange("(t p k) c -> t p k c", t=f8, p=128, k=64)
                  if f8 > 0
                  else None
              )
              d0bv = (
                  d0b[:].rearrange("(t p k) c -> t p k c", t=4 - f8, p=128, k=64)
                  if f8 < 4
                  else None
              )
              ld0s = []
              for t in range(4):
                  is8 = t < f8
                  if mode == "compute":
                      ld = ldcs[t]
                  else:
                      ld = inp.tile(
                          [128, 64, C],
                          FP8 if is8 else BF16,
                          bufs=ld_bufs,
                          tag="ld8" if is8 else "ld",
                      )
                      dv = d0av[t] if is8 else d0bv[t - f8]
                      if d0_split == "all" or t == 0:
                          nc.sync.dma_start(
                              out=ld[:, 0:32, :], in_=dv[:, 0:32, :]
                          )
                          ringb_eng.dma_start(
                              out=ld[:, 32:64, :], in_=dv[:, 32:64, :]
                          )
                      else:
                          eng = nc.sync if t % 2 == 0 else ringb_eng
                          eng.dma_start(out=ld[:], in_=dv)
                  ld0s.append(ld)

              # ---- data1 / data2 loads ----
              ld1 = ld1c if mode == "compute" else inp.tile(
                  [128, 4, 8, C],
                  BF16 if d1_mode == "bf16" else FP8,
                  bufs=1,
                  tag="ld1",
              )
              ld2 = ld2c if mode == "compute" else inp.tile(
                  [64, 8, C], BF16, bufs=1, tag="ld2"
              )
              if mode != "compute":
                  d1v = d1[:].rearrange("(j p k) c -> p j k c", j=4, p=128, k=8)
                  nc.sync.dma_start(out=ld1[:, 0:2], in_=d1v[:, 0:2])
                  ringb_eng.dma_start(out=ld1[:, 2:4], in_=d1v[:, 2:4])
                  ringb_eng.dma_start(
                      out=ld2[:], in_=d2[:].rearrange("(p k) c -> p k c", p=64, k=8)
                  )

              if mode == "dma":
                  dummy = work.tile([1, 8], F32, tag="dummy")
                  for t in range(4):
                      nc.vector.tensor_copy(
                          dummy[:, t : t + 1], ld0s[t][0:1, 0, 0:1]
                      )
                  nc.vector.tensor_copy(dummy[:, 4:5], ld1[0:1, 0, 0, 0:1])
                  nc.vector.tensor_copy(dummy[:, 5:6], ld2[0:1, 0, 0:1])
                  nc.vector.tensor_copy(outs[:, 0:8], dummy[:, 0:8])
                  nc.gpsimd.dma_start(out=out[:], in_=outs[:])
                  continue

              # stage-1 pool: in-place contiguous tensor-tensor max tree per
              # tile (64 sibling rows -> 1 per partition), then PE transpose
              # and a single contiguous 8:1 reduce over d3 siblings.  fp8
              # tiles run the first level as a converting fp8->bf16 max (1x
              # DVE mode) into a bf16 scratch; later levels run 2x on bf16.
              for t in range(4):
                  if t < act:
                      # fp8 tile upconverted by the scalar engine (two half
                      # copies so each starts as soon as its DMA half lands);
                      # the DVE then runs the plain all-bf16 tree.
                      c8 = work.tile([128, 64, C], BF16, tag="c8", bufs=2)
                      nc.scalar.copy(c8[:, 0:32, :], ld0s[t][:, 0:32, :])
                      nc.scalar.copy(c8[:, 32:64, :], ld0s[t][:, 32:64, :])
                      src, n = c8, 4096
                  elif t < f8:
                      s = work.tile([128, 32, C], BF16, tag="s8", bufs=2)
                      nc.vector.tensor_tensor(
                          out=s[:],
                          in0=ld0s[t][:, 0:32, :],
                          in1=ld0s[t][:, 32:64, :],
                          op=mybir.AluOpType.max,
                      )
                      src, n = s, 2048
                  else:
                      src, n = ld0s[t], 4096
                  ldf = src[:].rearrange("p k c -> p (k c)")
                  while n >= C:
                      nc.vector.tensor_tensor(
                          out=ldf[:, 0:n],
                          in0=ldf[:, 0:n],
                          in1=ldf[:, n : 2 * n],
                          op=mybir.AluOpType.max,
                      )
                      n //= 2
                  nc.tensor.transpose(
                      tpb0[:, t * 128 : (t + 1) * 128], src[:, 0, :], ident[:]
                  )
              nc.vector.reduce_max(
                  x0[:], tpb0[:].rearrange("p (g k) -> p g k", k=8), axis=AX
              )

              # ---- data1: partition = d3 node (8 contiguous rows) ----
              if d1_mode == "bf16":
                  src1, shs = ld1, (4, 2, 1)
              elif d1_mode == "dve":
                  # converting fp8->bf16 first level (1x DVE mode)
                  src1 = work.tile([128, 4, 4, C], BF16, tag="s1", bufs=2)
                  nc.vector.tensor_tensor(
                      out=src1[:],
                      in0=ld1[:, :, 0:4, :],
                      in1=ld1[:, :, 4:8, :],
                      op=mybir.AluOpType.max,
                  )
                  shs = (2, 1)
              else:  # "act": scalar-engine upconvert, plain bf16 tree
                  src1 = work.tile([128, 4, 8, C], BF16, tag="s1a", bufs=2)
                  nc.scalar.copy(src1[:, 0:2], ld1[:, 0:2])
                  nc.scalar.copy(src1[:, 2:4], ld1[:, 2:4])
                  shs = (4, 2, 1)
              for sh in shs:
                  nc.vector.tensor_tensor(
                      out=src1[:, :, 0:sh, :],
                      in0=src1[:, :, 0:sh, :],
                      in1=src1[:, :, sh : 2 * sh, :],
                      op=mybir.AluOpType.max,
                  )
              for j in range(4):
                  nc.tensor.transpose(
                      tpb1[:, j * 128 : (j + 1) * 128], src1[:, j, 0, :], ident[:]
                  )
              nc.vector.reduce_max(
                  x1[:], tpb1[:].rearrange("p (g k) -> p g k", k=8), axis=AX
              )

              # ---- data2: partition = d2 node (8 contiguous rows) ----
              for sh in (4, 2, 1):
                  nc.vector.tensor_tensor(
                      out=ld2[:, 0:sh, :],
                      in0=ld2[:, 0:sh, :],
                      in1=ld2[:, sh : 2 * sh, :],
                      op=mybir.AluOpType.max,
                  )
              tp2 = pt.tile([128, 64], BF16, tag="tp", bufs=1)
              nc.tensor.transpose(tp2[:], ld2[:, 0, :], ident[0:64, 0:64])
              nc.scalar.copy(x2[:], tp2[:])

              if mode == "pool":
                  nc.vector.tensor_copy(outs[:, 0:1], x0[0:1, 0:1])
                  nc.vector.tensor_copy(outs[:, 1:2], x1[0:1, 0:1])
                  nc.vector.tensor_copy(outs[:, 2:3], x2[0:1, 0:1])
                  nc.gpsimd.dma_start(out=out[:], in_=outs[:])
                  continue

              # Conv matmuls: 8 m-chunks x 3 K-chunks; accumulation groups
              # run back-to-back within each bank.  ScalarE activation
              # (bias + relu + node-sum via accum_out) drains each slice.
              for m in range(8):
                  # parity-interleaved banks: ScalarE drains slice m in one
                  # bank while PE runs group m+1's matmuls in the other
                  pybank = psum_ya if m % 2 == 0 else psum_yb
                  sl = pybank[:, (m // 2) * 64 : (m // 2 + 1) * 64]
                  for ki, xk in ((0, x0), (1, x1), (2, x2)):
                      nc.tensor.matmul(
                          sl,
                          wT_s[:, ki * OUTC + m * 128 : ki * OUTC + (m + 1) * 128],
                          xk[:],
                          start=(ki == 0),
                          stop=(ki == 2),
                      )
                  nc.scalar.activation(
                      yscr[:],
                      sl,
                      mybir.ActivationFunctionType.Relu,
                      bias=bias8_s[:, m : m + 1],
                      scale=1.0,
                      accum_out=stile[:, m : m + 1],
                  )

              for m in range(8):
                  nc.tensor.matmul(
                      psum_l[:],
                      stile[:, m : m + 1],
                      hw8_s[:, m * NCLS : (m + 1) * NCLS],
                      start=(m == 0),
                      stop=False,
                  )
              # head_b folded in as a rank-1 (K=1) accumulation: ones.T @ hb
              nc.tensor.matmul(
                  psum_l[:], ones1[:], hb_s[:], start=False, stop=True
              )
              nc.scalar.copy(outs[:], psum_l[:])
              nc.gpsimd.dma_start(out=out[:], in_=outs[:])

    if split:
        _split_multiwaits(nc)
    return nc


_NC = None


def _get_nc():
    global _NC
    if _NC is None:
        _NC = _build_nc()
    return _NC


def make_in_maps(
    data0, data1, data2, conv_w, bn_gamma, bn_beta, bn_mean, bn_var, head_w, head_b
):
    import ml_dtypes

    f = np.float32
    bf = ml_dtypes.bfloat16
    f8 = ml_dtypes.float8_e4m3
    rows8 = F8_TILES * 8192  # per-core data0 rows streamed as fp8
    d0c = np.asarray(data0, dtype=f).reshape(N_CORES, D0, C)
    data0a = np.ascontiguousarray(d0c[:, :rows8], dtype=f8)
    data0b = np.ascontiguousarray(d0c[:, rows8:], dtype=bf)
    data1 = np.ascontiguousarray(
        np.asarray(data1, dtype=f), dtype=bf if D1_MODE == "bf16" else f8
    )
    data2 = np.ascontiguousarray(np.asarray(data2, dtype=f), dtype=bf)
    conv_w = np.asarray(conv_w, dtype=f)
    bn_gamma = np.asarray(bn_gamma, dtype=f)
    bn_beta = np.asarray(bn_beta, dtype=f)
    bn_mean = np.asarray(bn_mean, dtype=f)
    bn_var = np.asarray(bn_var, dtype=f)
    head_w = np.asarray(head_w, dtype=f)
    head_b = np.asarray(head_b, dtype=f)

    inv = (bn_gamma / np.sqrt(bn_var + np.float32(1e-5))).astype(f)
    w_folded = (conv_w * inv[None, :]).astype(f)  # [384, 1024]
    b_folded = (bn_beta - bn_mean * inv).astype(f)  # [1024]

    # wT[p, k*1024+j] = W'[k*128+p, j]  (K-chunk-major along free dim)
    wT = np.ascontiguousarray(
        w_folded.reshape(3, 128, OUTC).transpose(1, 0, 2).reshape(128, 3 * OUTC),
        dtype=bf,
    )
    # bias8[p, m] = b'[m*128+p]
    bias8 = np.ascontiguousarray(b_folded.reshape(8, 128).T)
    # hw8[p, m*40+q] = head_w[m*128+p, q] / 64   (1/64 folds the mean-pool)
    hw8 = np.ascontiguousarray(
        (head_w / np.float32(N2)).reshape(8, 128, NCLS).transpose(1, 0, 2).reshape(128, 8 * NCLS)
    )
    hb = np.ascontiguousarray(head_b.reshape(1, NCLS))

    in_maps = []
    for c in range(N_CORES):
        m = {
            "d1": data1[c * D1 : (c + 1) * D1],
            "d2": data2[c * D2 : (c + 1) * D2],
            "wT": wT,
            "bias8": bias8,
            "hw8": hw8,
            "hb": hb,
        }
        if F8_TILES > 0:
            m["d0a"] = data0a[c]
        if F8_TILES < 4:
            m["d0b"] = data0b[c]
        in_maps.append(m)
    return in_maps


_RUNNER = None


def _make_runner(nc):
    """Jitted SPMD executor (mirrors bass2jax.run_bass_via_pjrt but reuses
    one jit so repeated calls don't re-trace/re-compile)."""
    if True:
        import jax
        from jax.experimental.shard_map import shard_map
        from jax.sharding import Mesh, PartitionSpec

        from concourse import bass2jax, mybir as mb

        bass2jax.install_neuronx_cc_hook()
        partition_name = (
            nc.partition_id_tensor.name if nc.partition_id_tensor else None
        )
        in_names, out_names, out_avals, zero_outs = [], [], [], []
        for alloc in nc.m.functions[0].allocations:
            if not isinstance(alloc, mb.MemoryLocationSet):
                continue
            name = alloc.memorylocations[0].name
            if alloc.kind == "ExternalInput":
                if name != partition_name:
                    in_names.append(name)
            elif alloc.kind == "ExternalOutput":
                out_names.append(name)
                shape = tuple(alloc.tensor_shape)
                dtype = mb.dt.np(alloc.dtype)
                out_avals.append(jax.core.ShapedArray(shape, dtype))
                zero_outs.append(np.zeros(shape, dtype))
        n_params = len(in_names)
        all_in_names = in_names + out_names
        if partition_name is not None:
            all_in_names = all_in_names + [partition_name]

        def _body(*args):
            operands = list(args)
            if partition_name is not None:
                operands.append(bass2jax.partition_id_tensor())
            outs = bass2jax._bass_exec_p.bind(
                *operands,
                out_avals=tuple(out_avals),
                in_names=tuple(all_in_names),
                out_names=tuple(out_names),
                lowering_input_output_aliases=(),
                sim_require_finite=True,
                sim_require_nnan=True,
                nc=nc,
            )
            return tuple(outs)

        devices = jax.devices()[:N_CORES]
        mesh = Mesh(np.asarray(devices), ("core",))
        n_outs = len(out_avals)
        in_specs = (PartitionSpec("core"),) * (n_params + n_outs)
        out_specs = (PartitionSpec("core"),) * n_outs
        # No donation: the kernel writes every element of "out", so the
        # zero placeholder inputs can live on device and be reused.
        sharded = jax.jit(
            shard_map(
                _body,
                mesh=mesh,
                in_specs=in_specs,
                out_specs=out_specs,
                check_rep=False,
            ),
            keep_unused=True,
        )
        return dict(
            nc=nc,
            sharded=sharded,
            in_names=in_names,
            out_names=out_names,
            out_avals=out_avals,
            zero_outs=zero_outs,
            mesh=mesh,
        )


def _get_runner():
    global _RUNNER
    if _RUNNER is None:
        _RUNNER = _make_runner(_get_nc())
    return _RUNNER


def _concat_inputs(r, in_maps):
    return [
        np.concatenate([np.asarray(m[name]) for m in in_maps], axis=0)
        for name in r["in_names"]
    ]


def _concat_zeros(r):
    return [
        np.zeros((N_CORES * z.shape[0], *z.shape[1:]), z.dtype)
        for z in r["zero_outs"]
    ]


def _run(r, concat_in, concat_zeros=None):
    if concat_zeros is None:
        concat_zeros = _concat_zeros(r)
    out_arrs = r["sharded"](*concat_in, *concat_zeros)
    return out_arrs


def kernel(**inputs) -> np.ndarray:
    r = _get_runner()
    in_maps = make_in_maps(**inputs)
    out_arrs = _run(r, _concat_inputs(r, in_maps))
    return np.asarray(out_arrs[r["out_names"].index("out")])


def device_place_and_time(r, inputs, iters=20, batches=4):
    """Pre-place inputs on device, then time batches of back-to-back
    dispatches.  Returns (per-call seconds list, out array)."""
    import time

    import jax
    from jax.sharding import NamedSharding, PartitionSpec

    sharding = NamedSharding(r["mesh"], PartitionSpec("core"))
    concat_in = _concat_inputs(r, make_in_maps(**inputs))
    dev_in = [jax.device_put(a, sharding) for a in concat_in]
    dev_zeros = [jax.device_put(z, sharding) for z in _concat_zeros(r)]
    out_arrs = _run(r, dev_in, dev_zeros)  # warm
    out = np.asarray(out_arrs[r["out_names"].index("out")])
    times = []
    for _ in range(batches):
        t0 = time.perf_counter()
        last = None
        for _ in range(iters):
            last = _run(r, dev_in, dev_zeros)
        jax.block_until_ready(last)
        t1 = time.perf_counter()
        times.append((t1 - t0) / iters)
    return times, out

